# revision 38
# baseline (speedup 1.0000x reference)
"""Trainium2 Bass kernel for nn_DecoderBlockWithKeywords.

Decoder block: causal self-attn + gated (source-code / keywords) cross-attn
+ template cross-attn + FFN, with 4 LayerNorms.  B=4, T=1024, D=512, H=8,
dh=64, DFF=2048.

Sharding: pure data-parallel over (batch, query-half) -> 8 NeuronCores, no
collectives.  Each core holds all weights (fp16) and computes 512 query
tokens of one batch element.

Layout strategy: every activation lives feature-major (X^T: [D on
partitions, tokens on free]).  Host pre-transposes/casts inputs.  Q/K
projections are weight-stationary (out feature-major); V is produced
token-major via activation-stationary matmuls so the attention AV matmul
needs no transposes at all.  Scores are computed as S^T = K_h Q_h^T
([kv, q]); softmax runs without max-subtraction (logits are O(1); masked
lanes get -1e6 bias fused into the ACT exp).  Softmax denominators come
from a ones-column appended to V inside the same AV matmul; per-column
scales (softmax 1/n, LN mean/rstd, gate g) are broadcast across partitions
with a PE ones-outer-product into a free PSUM bank and applied by one DVE
op reading that PSUM operand.  LayerNorm is done feature-major: column sums
via PE ones-matmuls, rstd = exp(-0.5*ln(v)) on ACT (single activation-table
set, zero table switches).  Residuals follow the reference post-LN chaining
(z = LN(y + y2), z_end = LN(z + z2), out = LN(z_end + ff)).

Programs are specialized at build time to the actual kv lengths (read from
the int32 length inputs), so masked kv tiles are skipped entirely; up to 8
distinct programs (4 batches x even/odd query half) are compiled and
launched concurrently on disjoint device subsets.
"""

import os
import sys
import threading

import numpy as np

for _p in ("/opt/trn_rl_repo", "/root/.axon_site"):
    if os.path.isdir(_p) and _p not in sys.path:
        sys.path.append(_p)

import ml_dtypes
from contextlib import ExitStack

import concourse.bass as bass
import concourse.mybir as mybir
from concourse import bacc
from concourse.tile import TileContext

BF16 = np.float16
F32 = np.float32
NEG = -1000000.0
B, T, S, TM, KW, D, H, DFF = 4, 1024, 1024, 512, 64, 512, 8, 2048
DH = D // H  # 64
P = 128
NCH = D // P  # 4 feature chunks
AF = mybir.ActivationFunctionType
OP = mybir.AluOpType


# ---------------------------------------------------------------------------
# program builder
# ---------------------------------------------------------------------------

def build_program(qh, kts_cc, kts_ct, gate_b=0.0, apply_affine=False, debug=False):
    """Build one core's Bass program.

    qh: 0/1 query half.  kts_cc/kts_ct: number of 128-wide kv tiles for the
    source-code / template cross attentions (specialized to actual length).
    """
    f32, bf16 = mybir.dt.float32, mybir.dt.float16
    KV = 512 * (qh + 1)          # self-attn kv range
    QOFF = qh * 512              # q columns inside xkvT

    nc = bacc.Bacc("TRN2", target_bir_lowering=False, debug=False)

    def din(name, shape, dt=bf16):
        return nc.dram_tensor(name, shape, dt, kind="ExternalInput").ap()

    fp8 = mybir.dt.float8e4
    xkv8T = din("xkv8T", [D, KV], fp8)
    src8T = din("src8T", [D, kts_cc * P], fp8)
    tmpl8T = din("tmpl8T", [D, kts_ct * P], fp8)
    kw8T = din("kw8T", [D, KW], fp8)
    xqT = din("xqT", [D, 512])
    W8NAMES = ("sa_wk", "sa_wv", "sa_wq", "cc_wk", "cc_wv",
               "ck_wk", "ck_wv", "ct_wk", "ct_wv")
    wnames = [f"{n}_{p}" for n in ("sa", "cc", "ct", "ck")
              for p in ("wq", "wk", "wv", "wo")]
    wd = {n: din(n, [D, D], fp8 if n in W8NAMES else bf16)
          for n in wnames}
    w1d = din("ffn_w1", [D, DFF])
    w2d = din("ffn_w2", [DFF, D])
    gwA = din("gwA", [D, 1])
    gwB = din("gwB", [D, 1])
    staird = din("stair", [P, P])
    ccbias_d = din("cc_bias", [P, 1], f32)
    ctbias_d = din("ct_bias", [P, 1], f32)
    kwbias_d = din("kw_bias", [KW, 1], f32)
    affine_d = din("ln_affine", [P, NCH * 8], f32) if apply_affine else None
    outT = nc.dram_tensor("outT", [D, 512], bf16, kind="ExternalOutput").ap()
    dbg_outs = {}

    def mkdbg(nm, shape):
        if nm not in dbg_outs:
            dbg_outs[nm] = nc.dram_tensor(f"dbg_{nm}", shape, f32,
                                          kind="ExternalOutput").ap()
        return dbg_outs[nm]

    with TileContext(nc, pool_alloc_mode="queue") as tc, ExitStack() as ctx:
        # Pin the activation table to natural_log_exp_and_others (set 6):
        # it contains exp/ln/square/relu/copy/identity, i.e. every ACT
        # function this program uses, so no further table loads are needed.
        nc.scalar.add_instruction(mybir.InstLoadActFuncSet(
            name=nc.get_next_instruction_name(), act_func_set_id=6,
            ins=[], outs=[]))
        pers = ctx.enter_context(tc.tile_pool(name="pers", bufs=1))
        # ---- persistent small constants -------------------------------
        stair = pers.tile([P, P], bf16, name="stair_t")
        nc.sync.dma_start(out=stair, in_=staird)
        # selector for the head-pair 1/n broadcast: row 0 -> partitions
        # 0:64 (even head), row 32 -> partitions 64:128 (odd head)
        # head-pair 1/n machinery: denominators for pairs (0,1) land on
        # rows 0/32/64/96 of tile A, pairs (2,3) on tile B (32-aligned
        # partition writes only).  One DVE reciprocal+cast per tile.
        # selAB col block hp%2 maps rows (0,32) or (64,96) onto the
        # 64-partition halves of a pair's output.
        selAB = pers.tile([97, 2 * P], bf16, name="selAB_t")
        nc.vector.memset(selAB, 0.0)
        nc.gpsimd.memset(selAB[0:1, 0:DH], 1.0)
        nc.gpsimd.memset(selAB[32:33, DH:P], 1.0)
        nc.gpsimd.memset(selAB[64:65, P:P + DH], 1.0)
        nc.gpsimd.memset(selAB[96:97, P + DH:2 * P], 1.0)
        nden97 = [pers.tile([97, 512], f32, name=f"nden97_{i}")
                  for i in range(2)]
        for t in nden97:
            nc.vector.memset(t, 1.0)
        nrec97 = [pers.tile([97, 512], f32, name=f"nrec97_{i}")
                  for i in range(2)]
        ninv97 = [pers.tile([97, 512], bf16, name=f"ninv97_{i}")
                  for i in range(2)]
        ccbias = pers.tile([P, 1], f32, name="ccbias_t")
        nc.sync.dma_start(out=ccbias, in_=ccbias_d)
        ctbias = pers.tile([P, 1], f32, name="ctbias_t")
        nc.sync.dma_start(out=ctbias, in_=ctbias_d)
        kwbias = pers.tile([KW, 1], f32, name="kwbias_t")
        nc.sync.dma_start(out=kwbias, in_=kwbias_d)
        gwa_t = pers.tile([P, NCH], bf16, name="gwa_t")
        nc.sync.dma_start(out=gwa_t, in_=gwA.rearrange("(i p) o -> p i o", p=P))
        gwb_t = pers.tile([P, NCH], bf16, name="gwb_t")
        nc.sync.dma_start(out=gwb_t, in_=gwB.rearrange("(i p) o -> p i o", p=P))
        ones_b = pers.tile([P, 1], bf16, name="ones_b")
        nc.vector.memset(ones_b, 1.0)
        ones_row = pers.tile([1, P], bf16, name="ones_row")
        nc.vector.memset(ones_row, 1.0)
        eps_t = pers.tile([1, 1], f32, name="eps_t")
        nc.vector.memset(eps_t, 1e-5)
        gb_t = pers.tile([1, 1], f32, name="gb_t")
        nc.vector.memset(gb_t, -float(gate_b))
        affine = None
        if apply_affine:
            affine = pers.tile([P, NCH * 8], f32, name="affine_t")
            nc.sync.dma_start(out=affine, in_=affine_d)

        def tap(nm, tiles):
            if not debug:
                return
            cols = tiles[0].shape[-1]
            d = mkdbg(nm, [len(tiles) * P, cols])
            for i, t in enumerate(tiles):
                rows = t.shape[0]
                nc.gpsimd.dma_start(out=d[i * P:i * P + rows, :], in_=t)

        # ---- global shared pools --------------------------------------
        # residual/LN-out tiles, reused across stages via shared tags
        rpool = ctx.enter_context(tc.tile_pool(name="rpool", bufs=1))

        def mktiles(nm, cols=512, dt=f32, n=NCH, tagp=None):
            tagp = tagp or nm
            return [rpool.tile([P, cols], dt, name=f"{nm}{i}", tag=f"{tagp}{i}",
                               bufs=1) for i in range(n)]

        # small 1/8-partition tiles + broadcast tiles, shared by all stages
        smallp = ctx.enter_context(tc.tile_pool(name="smallp", bufs=1))
        # transient [128, *] tiles (exp outputs, LN scratch, gate scratch)
        trp = ctx.enter_context(tc.tile_pool(name="trp", bufs=1))
        # PSUM: pps = projection/V accumulators; x_ps = paired scores
        # (2 banks each); x_po = AV out + LN stats + gate
        psA = ctx.enter_context(tc.tile_pool(name="psA", bufs=2, space="PSUM"))
        psB = ctx.enter_context(tc.tile_pool(name="psB", bufs=2, space="PSUM"))

        def load_w(pool, names):
            for n in names:
                if n in W8NAMES:
                    fp8 = mybir.dt.float8e4
                    wt[n] = pool.tile([P, 4 * D], fp8, name=f"{n}_t",
                                      tag=f"{n}_t", bufs=1)
                    nc.sync.dma_start(
                        out=wt[n].rearrange("p (c o n) -> p c o n",
                                            c=2, o=2),
                        in_=wd[n].rearrange("(c o p) n -> p c o n",
                                            c=2, o=2, p=P))
                else:
                    wt[n] = pool.tile([P, NCH * D], bf16, name=f"{n}_t",
                                      tag=f"{n}_t", bufs=1)
                    nc.sync.dma_start(
                        out=wt[n].rearrange("p (i n) -> p i n", n=D),
                        in_=wd[n].rearrange("(i p) n -> p i n", p=P))
        wt = {}

        def w8_lhsT(n, c, j):
            return wt[n].rearrange("p (c o n) -> p c o n",
                                   c=2, o=2)[:, c, :, j * P:(j + 1) * P]

        def w8_rhs(n, c):
            return wt[n].rearrange("p (c o n) -> p c o n", c=2, o=2)[:, c]

        def w8_flat(n, c, o, j):
            return wt[n].rearrange("p (c o n) -> p c o n",
                                   c=2, o=2)[:, c, o, j * P:(j + 1) * P]

        def load_act8(pool, nm, dram_ap, cols):
            fp8 = mybir.dt.float8e4
            tiles = []
            for c in range(2):
                t = pool.tile([P, 2 * cols], fp8, name=f"{nm}{c}",
                              tag=f"{nm}{c}", bufs=1)
                nc.sync.dma_start(
                    out=t.rearrange("p (o n) -> p o n", o=2),
                    in_=dram_ap[c * 2 * P:(c + 1) * 2 * P, :].rearrange(
                        "(o p) n -> p o n", o=2, p=P))
                tiles.append(t.rearrange("p (o n) -> p o n", o=2))
            return tiles

        def w_lhsT(n, i, j):
            return wt[n][:, i * D + j * P: i * D + (j + 1) * P]

        def w_rhs(n, i, cols=D):
            return wt[n][:, i * D: i * D + cols]

        def load_act(pool, nm, dram_ap, cols):
            tiles = []
            for i in range(NCH):
                t = pool.tile([P, cols], bf16, name=f"{nm}{i}",
                              tag=f"{nm}{i}", bufs=1)
                nc.sync.dma_start(out=t, in_=dram_ap[i * P:(i + 1) * P, :])
                tiles.append(t)
            return tiles

        # ----------------------------------------------------------------
        # helpers
        # ----------------------------------------------------------------
        def proj_fm_groups(wn, rhs_tiles, ncols, out_tiles, evict):
            """Per-psum-group closures for a feature-major projection; each
            emits 4 accumulating matmuls + one eviction."""
            ntt = (ncols + 511) // 512
            groups = []
            for j in range(NCH):
                for t in range(ntt):
                    def g(j=j, t=t):
                        cs = t * 512
                        ce = min(ncols, cs + 512)
                        ps = psA.tile([P, ce - cs], mybir.dt.float32,
                                      name="proj_ps", tag="pps")
                        for i in range(NCH):
                            nc.tensor.matmul(ps, w_lhsT(wn, i, j),
                                             rhs_tiles[i][:, cs:ce],
                                             start=(i == 0),
                                             stop=(i == NCH - 1))
                        evict(j, cs, ce, ps, out_tiles)
                    groups.append(g)
            return groups

        def proj_fm(wn, rhs_tiles, ncols, out_tiles, evict):
            for g in proj_fm_groups(wn, rhs_tiles, ncols, out_tiles, evict):
                g()

        DR = mybir.MatmulPerfMode.DoubleRow
        DESC = 1.0 / 1024.0  # descale: activation x16, weight x64

        def evict_ds(j, cs, ce, ps, out_tiles, balance=False):
            if balance and (j + cs // 512) % 2 == 1:
                nc.scalar.mul(out_tiles[j][:, cs:ce], ps, DESC)
            else:
                nc.vector.tensor_scalar_mul(out_tiles[j][:, cs:ce], ps, DESC)

        def proj_fm8_groups(wn, rhs8, ncols, out_tiles, balance=False):
            """fp8 DoubleRow feature-major projection (descaled evict).
            Falls back to normal-mode fp8 matmuls when ncols < 128."""
            ntt = (ncols + 511) // 512
            groups = []
            for j in range(NCH):
                for t in range(ntt):
                    def g(j=j, t=t):
                        cs = t * 512
                        ce = min(ncols, cs + 512)
                        ps = psA.tile([P, ce - cs], mybir.dt.float32,
                                      name="proj_ps", tag="pps")
                        if ncols >= P:
                            for c in range(2):
                                nc.tensor.matmul(
                                    ps, w8_lhsT(wn, c, j),
                                    rhs8[c][:, :, cs:ce],
                                    start=(c == 0), stop=(c == 1),
                                    perf_mode=DR)
                        else:
                            for ci in range(4):
                                c, o = ci // 2, ci % 2
                                nc.tensor.matmul(
                                    ps, w8_flat(wn, c, o, j),
                                    rhs8[c][:, o, cs:ce],
                                    start=(ci == 0), stop=(ci == 3))
                        evict_ds(j, cs, ce, ps, out_tiles, balance)
                    groups.append(g)
            return groups

        def proj_v8_groups(enc8, wn, nkv, vt_list, vpool, ktag):
            nch_tok = (nkv + P - 1) // P
            vt_list.extend(
                vpool.tile([min(P, nkv - m * P), H * (DH + 1)], bf16,
                           name=f"{ktag}_v{m}", tag=f"{ktag}_v{m}", bufs=1)
                for m in range(nch_tok))
            groups = []
            for m in range(nch_tok):
                def g(m=m):
                    rows = vt_list[m].shape[0]
                    ps = psA.tile([rows, D], mybir.dt.float32,
                                  name="v_ps", tag="pps")
                    for c in range(2):
                        nc.tensor.matmul(ps,
                                         enc8[c][:, :, m * P:m * P + rows],
                                         w8_rhs(wn, c),
                                         start=(c == 0), stop=(c == 1),
                                         perf_mode=DR)
                    vt = vt_list[m]
                    src3 = ps.rearrange("p (g c) -> p g c", c=DH)
                    dst3 = vt.rearrange("p (g c) -> p g c", c=DH + 1)
                    nc.vector.tensor_scalar_mul(dst3[:, :, 0:DH], src3, DESC)
                    nc.gpsimd.memset(dst3[:, :, DH:DH + 1], 1.0)
                groups.append(g)
            return groups

        def evict_copy(j, cs, ce, ps, out_tiles):
            nc.vector.tensor_copy(out_tiles[j][:, cs:ce], ps)

        def evict_copy_bal(j, cs, ce, ps, out_tiles):
            if (j + cs // 512) % 2 == 0:
                nc.vector.tensor_copy(out_tiles[j][:, cs:ce], ps)
            else:
                nc.scalar.copy(out_tiles[j][:, cs:ce], ps)

        def proj_v_groups(enc_tiles, wn, nkv, vt_list, vpool, ktag):
            nch_tok = (nkv + P - 1) // P
            vt_list.extend(
                vpool.tile([min(P, nkv - m * P), H * (DH + 1)], bf16,
                           name=f"{ktag}_v{m}", tag=f"{ktag}_v{m}", bufs=1)
                for m in range(nch_tok))
            groups = []
            for m in range(nch_tok):
                def g(m=m):
                    rows = vt_list[m].shape[0]
                    ps = psA.tile([rows, D], mybir.dt.float32,
                                  name="v_ps", tag="pps")
                    for i in range(NCH):
                        nc.tensor.matmul(ps,
                                         enc_tiles[i][:, m * P:m * P + rows],
                                         w_rhs(wn, i),
                                         start=(i == 0), stop=(i == NCH - 1))
                    vt = vt_list[m]
                    src3 = ps.rearrange("p (g c) -> p g c", c=DH)
                    dst3 = vt.rearrange("p (g c) -> p g c", c=DH + 1)
                    nc.vector.tensor_copy(dst3[:, :, 0:DH], src3)
                    nc.gpsimd.memset(dst3[:, :, DH:DH + 1], 1.0)
                groups.append(g)
            return groups

        def proj_v(enc_tiles, wn, nkv, vt_list, vpool, ktag):
            for g in proj_v_groups(enc_tiles, wn, nkv, vt_list, vpool, ktag):
                g()

        def attention(qt, kt, vt_list, out_tiles, bias_tile, causal, ktag,
                      fillers=None):
            """Multi-head attention.  Head pairs share one [rows,1024]
            scores psum + one merged exp; the AV matmul for tile kt is
            emitted after the scores matmul for tile kt+1 so the ACT exp
            overlaps PE work.  Causal scores/AV are restricted to the
            unmasked column range.  The softmax normalization runs per
            head-pair (collect denominators on partitions 0/32, one DVE
            reciprocal, one K=33 selector broadcast, one fused multiply)
            so it hides under the next pair's kt loop.  `fillers` is a
            list of closures emitting independent PE work; one is popped
            after each kt iteration to fill the exp-wait bubbles."""
            nkt = len(vt_list)
            fillers = fillers if fillers is not None else []

            def pop_filler():
                if fillers:
                    fillers.pop(0)()

            for hp in range(H // 2):
                po = []
                for s in range(2):
                    po.append(psB.tile([DH + 1, 512], mybir.dt.float32,
                                       name=f"{ktag}_po{s}", tag="x_po"))
                pend = None  # deferred AV: (kt_i, pt2, c0)

                def flush_av(last):
                    kt_i, pt2, c0 = pend
                    for s in range(2):
                        h = 2 * hp + s
                        nc.tensor.matmul(
                            po[s][:, c0:512],
                            vt_list[kt_i][:, h * (DH + 1):
                                          (h + 1) * (DH + 1)],
                            pt2[:, s * 512 + c0:(s + 1) * 512],
                            start=(kt_i == 0), stop=last)

                for kt_i in range(nkt):
                    rows = vt_list[kt_i].shape[0]
                    d = kt_i - (nkt - 4) if causal else -1
                    c0 = d * P if (causal and d > 0) else 0
                    ps2 = psB.tile([rows, 1024], mybir.dt.float32,
                                   name=f"{ktag}_ps", tag="x_ps")
                    pt2 = trp.tile([rows, 1024], bf16,
                                   name=f"{ktag}_pt", tag="pt", bufs=3)
                    for s in range(2):
                        ro = s * DH
                        o = s * 512
                        nc.tensor.matmul(
                            ps2[:, o + c0:o + 512],
                            kt[hp][ro:ro + DH, kt_i * P:kt_i * P + rows],
                            qt[hp][ro:ro + DH, c0:512], start=True, stop=True)
                    if causal and d >= 0:
                        for s in range(2):
                            o = s * 512
                            nc.vector.tensor_add(
                                ps2[:, o + d * P:o + (d + 1) * P],
                                ps2[:, o + d * P:o + (d + 1) * P], stair)
                            nc.scalar.activation(pt2[:, o + c0:o + 512],
                                                 ps2[:, o + c0:o + 512],
                                                 AF.Exp, scale=0.125)
                    else:
                        bias = 0.0
                        if bias_tile is not None and kt_i == nkt - 1:
                            bias = bias_tile[:rows, :]
                        nc.scalar.activation(pt2, ps2, AF.Exp,
                                             bias=bias, scale=0.125)
                    if pend is not None:
                        flush_av(False)
                        if kt_i % 2 == 1:
                            pop_filler()
                    pend = (kt_i, pt2, c0)
                flush_av(True)
                # stage this pair's denominators into tile A (pairs 0,1)
                # or B (pairs 2,3) on 32-aligned partitions
                ab, r0 = hp // 2, (hp % 2) * DH
                nc.vector.tensor_copy(nden97[ab][r0:r0 + 1, :],
                                      po[0][DH:DH + 1, :])
                nc.vector.tensor_copy(nden97[ab][r0 + 32:r0 + 33, :],
                                      po[1][DH:DH + 1, :])
                nc.scalar.copy(out_tiles[hp][0:DH, :], po[0][0:DH, :])
                nc.vector.tensor_copy(out_tiles[hp][DH:P, :], po[1][0:DH, :])
                if hp % 2 == 1:
                    # both pairs of this tile staged: one reciprocal+cast,
                    # then normalize both pairs (overlaps the next loop)
                    nc.vector.reciprocal_approx_fast(out=nrec97[ab],
                                                     in_=nden97[ab])
                    nc.vector.tensor_copy(ninv97[ab], nrec97[ab])
                    for hq in (hp - 1, hp):
                        pop_filler()
                        nb = psA.tile([P, 512], mybir.dt.float32,
                                      name=f"{ktag}_nb{hq}", tag="pps")
                        nc.tensor.matmul(nb,
                                         selAB[:, (hq % 2) * P:
                                               (hq % 2 + 1) * P],
                                         ninv97[ab], start=True, stop=True)
                        nc.vector.tensor_mul(out_tiles[hq], out_tiles[hq],
                                             nb)
            for g in fillers:
                g()

        def layernorm(r_tiles, out_tiles, ln_idx, mid=None):
            sq = [trp.tile([P, 512], bf16, name=f"ln{ln_idx}_sq", tag="ln_sq",
                           bufs=2) for _ in range(NCH)]
            for j in range(NCH):
                nc.gpsimd.tensor_mul(sq[j], r_tiles[j], r_tiles[j])
            ps_s = psB.tile([1, 512], mybir.dt.float32,
                            name="ln_ps_s", tag="x_po")
            ps_q = psB.tile([1, 512], mybir.dt.float32,
                            name="ln_ps_q", tag="x_po")
            for j in range(NCH):
                nc.tensor.matmul(ps_s, ones_b, r_tiles[j],
                                 start=(j == 0), stop=(j == NCH - 1))
            for j in range(NCH):
                nc.tensor.matmul(ps_q, ones_b, sq[j],
                                 start=(j == 0), stop=(j == NCH - 1))
            if mid is not None:
                mid()
            mean16 = smallp.tile([1, 512], bf16,
                                 name="ln_mean16", tag="ln_stat", bufs=3)
            nc.vector.tensor_scalar_mul(mean16, ps_s, 1.0 / D)
            meanb = psB.tile([P, 512], mybir.dt.float32,
                             name="ln_meanb", tag="x_po")
            nc.tensor.matmul(meanb, ones_row, mean16, start=True, stop=True)
            msq = smallp.tile([1, 512], mybir.dt.float32,
                              name="ln_msq", tag="ln_stat", bufs=3)
            nc.scalar.activation(msq, ps_s, AF.Square, scale=1.0 / D)
            var = smallp.tile([1, 512], mybir.dt.float32,
                              name="ln_var", tag="ln_stat", bufs=3)
            nc.vector.scalar_tensor_tensor(var, ps_q, 1.0 / D, msq,
                                           op0=OP.mult, op1=OP.subtract)
            lnv = smallp.tile([1, 512], mybir.dt.float32,
                              name="ln_lnv", tag="ln_stat", bufs=3)
            nc.scalar.activation(lnv, var, AF.Ln, bias=eps_t[:, :])
            rstd = smallp.tile([1, 512], bf16,
                               name="ln_rstd", tag="ln_stat", bufs=3)
            nc.scalar.activation(rstd, lnv, AF.Exp, scale=-0.5)
            rstdb = psB.tile([P, 512], mybir.dt.float32,
                             name="ln_rstdb", tag="x_po")
            nc.tensor.matmul(rstdb, ones_row, rstd, start=True, stop=True)
            # stage the broadcasts to SBUF (bf16) so the applies can run
            # on the otherwise-idle GPSIMD engine (it cannot read PSUM)
            meanb_s = trp.tile([P, 512], bf16, name="ln_meanb_s",
                               tag="ln_mb", bufs=2)
            nc.vector.tensor_copy(meanb_s, meanb)
            rstdb_s = trp.tile([P, 512], bf16, name="ln_rstdb_s",
                               tag="ln_rb", bufs=2)
            nc.vector.tensor_copy(rstdb_s, rstdb)
            for j in range(NCH):
                tmp = trp.tile([P, 512], bf16,
                               name="ln_tmp", tag="ln_tmp", bufs=2)
                nc.gpsimd.tensor_sub(tmp, r_tiles[j], meanb_s)
                nc.gpsimd.tensor_mul(out_tiles[j], tmp, rstdb_s)
                if apply_affine:
                    g = affine[:, ln_idx * 2 * NCH + j: ln_idx * 2 * NCH + j + 1]
                    b = affine[:, ln_idx * 2 * NCH + NCH + j:
                               ln_idx * 2 * NCH + NCH + j + 1]
                    nc.vector.tensor_scalar(out_tiles[j], out_tiles[j],
                                            g, b, op0=OP.mult, op1=OP.add)

        # ================================================================
        # emission (ordered for cross-stage overlap)
        # ================================================================
        r1 = mktiles("r1", dt=bf16, tagp="rA")
        y = mktiles("y", dt=bf16, tagp="lnA")
        r2 = mktiles("r2", dt=bf16, tagp="rB")
        z = mktiles("z", dt=bf16, tagp="lnB")
        r3 = None  # allocated after r1 dies
        ze = None

        # ct pool created first so it outlives ccsb (LIFO pool stack);
        # its DMA loads are issued after LN1 and overlap the cc/ck stage
        ctsb = ctx.enter_context(tc.tile_pool(name="tail_sb", bufs=1))
        ccsb_cm = tc.tile_pool(name="cc_sb", bufs=1)
        ccsb = ccsb_cm.__enter__()
        sasb_cm = tc.tile_pool(name="sa_sb", bufs=1)
        sasb = sasb_cm.__enter__()

        # --- stage 1: self attention (fp8 DoubleRow K/V/Q) ---
        load_w(sasb, ["sa_wk"])
        xkv8 = load_act8(sasb, "xkv8", xkv8T, KV)
        xq = load_act(sasb, "xq", xqT, 512)
        load_w(sasb, ["sa_wv", "sa_wq", "sa_wo"])
        qt = [sasb.tile([P, 512], bf16, name=f"sa_q{i}", tag=f"sa_q{i}",
                        bufs=1) for i in range(NCH)]
        ktl = [sasb.tile([P, KV], bf16, name=f"sa_k{i}", tag=f"sa_k{i}",
                         bufs=1) for i in range(NCH)]
        xq8 = [t[:, :, QOFF:QOFF + 512] for t in xkv8]
        for g in proj_fm8_groups("sa_wk", xkv8, KV, ktl, balance=True):
            g()
        vts = []
        for g in proj_v8_groups(xkv8, "sa_wv", KV, vts, sasb, "sa"):
            g()
        for g in proj_fm8_groups("sa_wq", xq8, 512, qt, balance=True):
            g()
        at = [trp.tile([P, 512], bf16, name=f"sa_at{i}", tag=f"at{i}",
                       bufs=1) for i in range(NCH)]
        # cc K/V projections are independent of sa: interleave them into
        # sa's kt loops as PE fillers (their DMA loads were issued above)
        load_w(ccsb, ["cc_wk", "cc_wv", "ck_wk", "ck_wv",
                      "cc_wq", "ck_wq", "cc_wo", "ck_wo"])
        srcl8 = load_act8(ccsb, "src8", src8T, kts_cc * P)
        kwe8 = load_act8(ccsb, "kw8", kw8T, KW)
        cc_kt = [ccsb.tile([P, kts_cc * P], bf16, name=f"cc_k{i}",
                           tag=f"cc_k{i}", bufs=1) for i in range(NCH)]
        cc_vts = []
        sa_fill = (proj_fm8_groups("cc_wk", srcl8, kts_cc * P, cc_kt)
                   + proj_v8_groups(srcl8, "cc_wv", kts_cc * P, cc_vts,
                                    ccsb, "cc"))
        attention(qt, ktl, vts, at, None, True, "sa", fillers=sa_fill)

        def evict_resid_x(j, cs, ce, ps, out_tiles):
            nc.vector.tensor_add(out_tiles[j][:, cs:ce], ps, xq[j])
        tap("sa_at", at)
        proj_fm("sa_wo", at, 512, r1, evict_resid_x)
        tap("r1", r1)
        ck_kt = [ccsb.tile([P, KW], bf16, name=f"ck_k{i}", tag=f"ck_k{i}",
                           bufs=1) for i in range(NCH)]
        ck_vts = []

        def ln1_mid():
            for g in proj_fm8_groups("ck_wk", kwe8, KW, ck_kt):
                g()
            for g in proj_v8_groups(kwe8, "ck_wv", KW, ck_vts, ccsb, "ck"):
                g()
        layernorm(r1, y, 0, mid=ln1_mid)
        tap("y", y)
        sasb_cm.__exit__(None, None, None)

        # ct weight/activation DMA loads overlap the whole cc/ck stage
        load_w(ctsb, ["ct_wk", "ct_wv", "ct_wq", "ct_wo"])
        tmpl8 = load_act8(ctsb, "tmpl8", tmpl8T, kts_ct * P)

        # --- stage 2: cc + ck cross attention + gate ---
        cc_qt = [ccsb.tile([P, 512], bf16, name=f"cc_q{i}", tag=f"cc_q{i}",
                           bufs=1) for i in range(NCH)]
        proj_fm("cc_wq", y, 512, cc_qt, evict_copy_bal)
        cc_at = [trp.tile([P, 512], bf16, name=f"cc_at{i}", tag=f"at{i}",
                          bufs=1) for i in range(NCH)]
        ck_qt = [ccsb.tile([P, 512], bf16, name=f"ck_q{i}", tag=f"ck_q{i}",
                           bufs=1) for i in range(NCH)]
        cc_fill = proj_fm_groups("ck_wq", y, 512, ck_qt, evict_copy)
        attention(cc_qt, cc_kt, cc_vts, cc_at, ccbias, False, "cc",
                  fillers=cc_fill)
        ck_at = [trp.tile([P, 512], bf16, name=f"ck_at{i}", tag=f"ckat{i}",
                          bufs=1) for i in range(NCH)]
        y2c = [ccsb.tile([P, 512], bf16, name=f"y2c{i}", tag=f"y2c{i}",
                         bufs=1) for i in range(NCH)]
        ck_fill = proj_fm_groups("cc_wo", cc_at, 512, y2c, evict_copy)
        attention(ck_qt, ck_kt, ck_vts, ck_at, kwbias, False, "ck",
                  fillers=ck_fill)
        y2k = [ccsb.tile([P, 512], bf16, name=f"y2k{i}", tag=f"y2k{i}",
                         bufs=1) for i in range(NCH)]
        proj_fm("ck_wo", ck_at, 512, y2k, evict_copy_bal)

        # --- gate ---
        ps_g = psB.tile([1, 512], mybir.dt.float32, name="gate_ps",
                        tag="x_po")
        for i in range(NCH):
            nc.tensor.matmul(ps_g, gwa_t[:, i:i + 1], y2c[i],
                             start=(i == 0), stop=False)
        for i in range(NCH):
            nc.tensor.matmul(ps_g, gwb_t[:, i:i + 1], y2k[i],
                             start=False, stop=(i == NCH - 1))
        # g-independent combine pieces, overlap the gate ACT/DVE chain
        gdt = [trp.tile([P, 512], bf16, name=f"gate_dt{j}", tag=f"gate_dt{j}",
                        bufs=1) for j in range(NCH)]
        for j in range(NCH):
            nc.gpsimd.tensor_sub(gdt[j], y2c[j], y2k[j])
            nc.gpsimd.tensor_add(r2[j], y[j], y2k[j])
        ct_kt = [ctsb.tile([P, kts_ct * P], bf16, name=f"ct_k{i}",
                           tag=f"ct_k{i}", bufs=1) for i in range(NCH)]

        def ct_mid():
            for g in proj_fm8_groups("ct_wk", tmpl8, kts_ct * P, ct_kt):
                g()
        ge = smallp.tile([1, 512], mybir.dt.float32, name="gate_e",
                         tag="gate_edg", bufs=2)
        nc.scalar.activation(ge, ps_g, AF.Exp, scale=-1.0, bias=gb_t[:, :])
        gp1 = smallp.tile([1, 512], mybir.dt.float32, name="gate_p1",
                          tag="gate_edg", bufs=2)
        nc.vector.tensor_scalar_add(gp1, ge, 1.0)
        grc = smallp.tile([1, 512], mybir.dt.float32, name="gate_rc",
                          tag="gate_edg", bufs=2)
        nc.vector.reciprocal_approx_fast(out=grc, in_=gp1)
        gg = smallp.tile([1, 512], bf16, name="gate_g",
                         tag="gate_edg", bufs=2)
        nc.vector.tensor_copy(gg, grc)
        ct_mid()
        ggb = psB.tile([P, 512], mybir.dt.float32, name="gate_gb",
                       tag="x_po")
        nc.tensor.matmul(ggb, ones_row, gg, start=True, stop=True)
        ggb_s = trp.tile([P, 512], bf16, name="gate_gb_s", tag="ln_mb",
                         bufs=2)
        nc.vector.tensor_copy(ggb_s, ggb)
        # r2 = (y + y2k) + g*(y2c - y2k)
        for j in range(NCH):
            nc.gpsimd.tensor_mul(gdt[j], gdt[j], ggb_s)
            nc.gpsimd.tensor_add(r2[j], r2[j], gdt[j])
        tap("y2c", y2c)
        tap("y2k", y2k)
        tap("r2", r2)
        ccsb_cm.__exit__(None, None, None)
        # FFN weights: DMA overlaps the ct attention stage
        ffsb = ctx.enter_context(tc.tile_pool(name="ff_sb", bufs=1))
        w1t = ffsb.tile([P, NCH * DFF], bf16, name="w1_t", tag="w1_t")
        nc.sync.dma_start(out=w1t.rearrange("p (i n) -> p i n", n=DFF),
                          in_=w1d.rearrange("(i p) n -> p i n", p=P))
        w2t = ffsb.tile([P, (DFF // P) * D], bf16, name="w2_t", tag="w2_t")
        nc.sync.dma_start(out=w2t.rearrange("p (i n) -> p i n", n=D),
                          in_=w2d.rearrange("(i p) n -> p i n", p=P))
        ct_vts = []

        def ln2_mid():
            for g in proj_v8_groups(tmpl8, "ct_wv", kts_ct * P, ct_vts,
                                    ctsb, "ct"):
                g()
        layernorm(r2, z, 1, mid=ln2_mid)
        tap("z", z)

        # --- stage 3: ct cross attention ---
        r3 = mktiles("r3", dt=bf16, tagp="rA")
        ze = mktiles("ze", dt=bf16, tagp="lnA")
        ct_qt = [ffsb.tile([P, 512], bf16, name=f"ct_q{i}", tag=f"ct_q{i}",
                           bufs=1) for i in range(NCH)]
        proj_fm("ct_wq", z, 512, ct_qt, evict_copy_bal)
        ct_at = [trp.tile([P, 512], bf16, name=f"ct_at{i}", tag=f"at{i}",
                          bufs=1) for i in range(NCH)]
        attention(ct_qt, ct_kt, ct_vts, ct_at, ctbias, False, "ct")

        def evict_resid_r2(j, cs, ce, ps, out_tiles):
            nc.vector.tensor_add(out_tiles[j][:, cs:ce], ps, z[j])
        tap("ct_at", ct_at)
        proj_fm("ct_wo", ct_at, 512, r3, evict_resid_r2)
        tap("r3", r3)
        layernorm(r3, ze, 2)
        tap("ze", ze)

        # --- stage 4: FFN ---
        ht = [ffsb.tile([P, 512], bf16, name=f"ff_h{i}", tag=f"ff_h{i}",
                        bufs=1) for i in range(DFF // P)]
        for jf in range(DFF // P):
            ps = psA.tile([P, 512], mybir.dt.float32, name="ff_ps",
                          tag="pps")
            for i in range(NCH):
                nc.tensor.matmul(ps, w1t[:, i * DFF + jf * P:
                                         i * DFF + (jf + 1) * P],
                                 ze[i], start=(i == 0), stop=(i == NCH - 1))
            if jf % 2 == 0:
                nc.scalar.activation(ht[jf], ps, AF.Relu)
            else:
                nc.vector.tensor_scalar_max(ht[jf], ps, 0.0)
        r4 = mktiles("r4", dt=bf16, tagp="rB")
        for j in range(NCH):
            ps = psA.tile([P, 512], mybir.dt.float32, name="ff_ps2",
                          tag="pps")
            for i in range(DFF // P):
                nc.tensor.matmul(ps, w2t[:, i * D + j * P: i * D + (j + 1) * P],
                                 ht[i], start=(i == 0),
                                 stop=(i == DFF // P - 1))
            nc.vector.tensor_add(r4[j], ps, ze[j])
        fin = [trp.tile([P, 512], bf16, name=f"fin{i}",
                        tag=f"at{i}", bufs=1) for i in range(NCH)]
        layernorm(r4, fin, 3)
        for j in range(NCH):
            nc.sync.dma_start(out=outT[j * P:(j + 1) * P, :], in_=fin[j])

    nc.compile()
    return nc


# ---------------------------------------------------------------------------
# host-side input preparation
# ---------------------------------------------------------------------------

W8NAMES_H = ("sa_wk", "sa_wv", "sa_wq", "cc_wk", "cc_wv",
             "ck_wk", "ck_wv", "ct_wk", "ct_wv")
FP8 = ml_dtypes.float8_e4m3
SW8, SX8 = 64.0, 16.0


def _prep_shared(inputs):
    """Cast/transform weights shared by every core."""
    sh = {}
    for n in ("sa", "cc", "ct", "ck"):
        for p in ("wq", "wk", "wv", "wo"):
            nm = f"{n}_{p}"
            if nm in W8NAMES_H:
                sh[nm] = np.ascontiguousarray(
                    np.clip(inputs[nm].astype(F32) * SW8,
                            -240, 240).astype(FP8))
            else:
                sh[nm] = np.ascontiguousarray(inputs[nm].astype(BF16))
    sh["ffn_w1"] = np.ascontiguousarray(inputs["ffn_w1"].astype(BF16))
    sh["ffn_w2"] = np.ascontiguousarray(inputs["ffn_w2"].astype(BF16))
    gw = inputs["gate_w"].astype(F32)
    sh["gwA"] = np.ascontiguousarray(gw[:D].astype(BF16))
    sh["gwB"] = np.ascontiguousarray(gw[D:].astype(BF16))
    kl, ql = np.arange(P)[:, None], np.arange(P)[None, :]
    sh["stair"] = np.where(kl <= ql, 0.0, NEG).astype(BF16)
    return sh


def _len_bias(L, kts, width=P):
    """[width,1] f32 additive bias for the LAST kv tile."""
    base = (kts - 1) * P
    idx = base + np.arange(width)
    return np.where(idx < L, 0.0, NEG).astype(F32)[:, None]


def _q8(a):
    return np.clip(a.astype(F32) * SX8, -240, 240).astype(FP8)


def _prep_core(inputs, sh, b, qh, kts_cc, kts_ct):
    KVn = 512 * (qh + 1)
    m = dict(sh)
    xT = inputs["x"][b].T.astype(F32)  # [D, T]
    m["xkv8T"] = np.ascontiguousarray(_q8(xT[:, :KVn]))
    m["xqT"] = np.ascontiguousarray(
        xT[:, qh * 512:(qh + 1) * 512].astype(BF16))
    Ls = int(inputs["source_code_len"][b])
    st = np.zeros((D, kts_cc * P), FP8)
    st[:, :Ls] = _q8(inputs["source_code_enc"][b, :Ls].T)
    m["src8T"] = st
    Lt = int(inputs["template_len"][b])
    tt = np.zeros((D, kts_ct * P), FP8)
    tt[:, :Lt] = _q8(inputs["template_enc"][b, :Lt].T)
    m["tmpl8T"] = tt
    m["kw8T"] = np.ascontiguousarray(_q8(inputs["keywords_enc"][b].T))
    m["cc_bias"] = _len_bias(Ls, kts_cc)
    m["ct_bias"] = _len_bias(Lt, kts_ct)
    m["kw_bias"] = _len_bias(int(inputs["keywords_len"][b]), 1, KW)
    return m


# ---------------------------------------------------------------------------
# concurrent multi-program PJRT runner (adapted from bass2jax.run_bass_via_pjrt)
# ---------------------------------------------------------------------------

def _run_groups(groups):
    """groups: list of (nc, core_ids, in_maps).  Dispatch all groups onto
    their own device subsets, then gather.  Returns {core_id: {name: arr}}."""
    import jax
    import numpy as _np
    from jax.sharding import Mesh, PartitionSpec
    from jax.experimental.shard_map import shard_map
    from concourse import bass2jax
    from concourse.bass2jax import (_bass_exec_p, install_neuronx_cc_hook,
                                    partition_id_tensor)

    install_neuronx_cc_hook()
    devices = jax.devices()

    def make_launch(nc, core_ids, in_maps):
        pname = (nc.partition_id_tensor.name
                 if nc.partition_id_tensor else None)
        in_names, out_names, out_avals, zero_outs = [], [], [], []
        for alloc in nc.m.functions[0].allocations:
            if not isinstance(alloc, mybir.MemoryLocationSet):
                continue
            name = alloc.memorylocations[0].name
            if alloc.kind == "ExternalInput":
                if name == pname:
                    continue
                in_names.append(name)
            elif alloc.kind == "ExternalOutput":
                shape = tuple(alloc.tensor_shape)
                dtype = mybir.dt.np(alloc.dtype)
                out_names.append(name)
                out_avals.append(jax.core.ShapedArray(shape, dtype))
                zero_outs.append(_np.zeros(shape, dtype))
        n_params, n_outs = len(in_names), len(out_avals)
        all_in_names = in_names + out_names
        if pname is not None:
            all_in_names = all_in_names + [pname]

        def _body(*args):
            operands = list(args)
            if pname is not None:
                operands.append(partition_id_tensor())
            outs = _bass_exec_p.bind(
                *operands, out_avals=tuple(out_avals),
                in_names=tuple(all_in_names), out_names=tuple(out_names),
                lowering_input_output_aliases=(),
                sim_require_finite=False, sim_require_nnan=False, nc=nc)
            return tuple(outs)

        donate = tuple(range(n_params, n_params + n_outs))
        devs = [devices[c] for c in core_ids]
        if len(core_ids) == 1:
            fn = jax.jit(_body, donate_argnums=donate, keep_unused=True,
                         device=devs[0])
            args = [in_maps[0][nm] for nm in in_names] + list(zero_outs)
            out_arrs = fn(*args)
            return out_names, out_avals, out_arrs, None
        mesh = Mesh(_np.asarray(devs), ("core",))
        in_specs = (PartitionSpec("core"),) * (n_params + n_outs)
        out_specs = (PartitionSpec("core"),) * n_outs
        fn = jax.jit(shard_map(_body, mesh=mesh, in_specs=in_specs,
                               out_specs=out_specs, check_rep=False),
                     donate_argnums=donate, keep_unused=True)
        cat = [_np.concatenate([_np.asarray(m[nm]) for m in in_maps], axis=0)
               for nm in in_names]
        catz = [_np.zeros((len(core_ids) * z.shape[0], *z.shape[1:]), z.dtype)
                for z in zero_outs]
        out_arrs = fn(*cat, *catz)
        return out_names, out_avals, out_arrs, len(core_ids)

    last_err = None
    for _attempt in range(3):
        try:
            launched = []
            for nc, core_ids, in_maps in groups:
                launched.append((core_ids, make_launch(nc, core_ids, in_maps)))
            results = {}
            for core_ids, (out_names, out_avals, out_arrs, ncores) in launched:
                if ncores is None:
                    results[core_ids[0]] = {nm: _np.asarray(out_arrs[i])
                                            for i, nm in enumerate(out_names)}
                else:
                    for ci, c in enumerate(core_ids):
                        results[c] = {
                            nm: _np.asarray(out_arrs[i]).reshape(
                                ncores, *out_avals[i].shape)[ci]
                            for i, nm in enumerate(out_names)}
            return results
        except Exception as e:  # transient NRT device errors: retry
            last_err = e
            import time as _time
            _time.sleep(2.0)
    raise last_err


_PROGRAM_CACHE = {}
_CACHE_LOCK = threading.Lock()


def _get_program(key):
    with _CACHE_LOCK:
        if key in _PROGRAM_CACHE:
            return _PROGRAM_CACHE[key]
    qh, kts_cc, kts_ct, gate_b, aff = key
    nc = build_program(qh, kts_cc, kts_ct, gate_b=gate_b, apply_affine=aff)
    with _CACHE_LOCK:
        _PROGRAM_CACHE[key] = nc
    return nc


# ---------------------------------------------------------------------------
# entry point
# ---------------------------------------------------------------------------

def kernel(**inputs):
    inputs = {k: np.asarray(v) for k, v in inputs.items()}
    gate_b = float(inputs["gate_b"].reshape(-1)[0])
    aff = not all(
        np.all(inputs[f"ln{j}_g"] == 1.0) and np.all(inputs[f"ln{j}_b"] == 0.0)
        for j in range(1, 5))
    affine_arr = None
    if aff:
        affine_arr = np.zeros((P, NCH * 8), F32)
        for ln in range(4):
            g = inputs[f"ln{ln + 1}_g"].astype(F32).reshape(NCH, P).T
            bb = inputs[f"ln{ln + 1}_b"].astype(F32).reshape(NCH, P).T
            affine_arr[:, ln * 2 * NCH: ln * 2 * NCH + NCH] = g
            affine_arr[:, ln * 2 * NCH + NCH: (ln + 1) * 2 * NCH] = bb

    sh = _prep_shared(inputs)
    # core -> (program key, in_map)
    core_keys, core_maps = [], []
    for c in range(8):
        b, qh = c // 2, c % 2
        kts_cc = max(1, -(-int(inputs["source_code_len"][b]) // P))
        kts_ct = max(1, -(-int(inputs["template_len"][b]) // P))
        key = (qh, kts_cc, kts_ct, gate_b, aff)
        m = _prep_core(inputs, sh, b, qh, kts_cc, kts_ct)
        if aff:
            m["ln_affine"] = affine_arr
        core_keys.append(key)
        core_maps.append(m)

    # build distinct programs (parallel threads: walrus compile is subprocess)
    distinct = sorted(set(core_keys))
    threads = [threading.Thread(target=_get_program, args=(k,))
               for k in distinct]
    for t in threads:
        t.start()
    for t in threads:
        t.join()

    groups = []
    for key in distinct:
        cores = [c for c in range(8) if core_keys[c] == key]
        groups.append((_get_program(key), cores, [core_maps[c] for c in cores]))

    results = _run_groups(groups)

    out = np.empty((B, T, D), np.float32)
    for c in range(8):
        b, qh = c // 2, c % 2
        out[b, qh * 512:(qh + 1) * 512, :] = results[c]["outT"].T
    return out



# revision 39
# speedup vs baseline: 1.0971x; 1.0971x over previous
"""Trainium2 Bass kernel for nn_DecoderBlockWithKeywords.

Decoder block: causal self-attn + gated (source-code / keywords) cross-attn
+ template cross-attn + FFN, with 4 LayerNorms.  B=4, T=1024, D=512, H=8,
dh=64, DFF=2048.

Sharding: pure data-parallel over (batch, query-half) -> 8 NeuronCores, no
collectives.  Each core holds all weights (fp16) and computes 512 query
tokens of one batch element.

Layout strategy: every activation lives feature-major (X^T: [D on
partitions, tokens on free]).  Host pre-transposes/casts inputs.  Q/K
projections are weight-stationary (out feature-major); V is produced
token-major via activation-stationary matmuls so the attention AV matmul
needs no transposes at all.  Scores are computed as S^T = K_h Q_h^T
([kv, q]); softmax runs without max-subtraction (logits are O(1); masked
lanes get -1e6 bias fused into the ACT exp).  Softmax denominators come
from a ones-column appended to V inside the same AV matmul; per-column
scales (softmax 1/n, LN mean/rstd, gate g) are broadcast across partitions
with a PE ones-outer-product into a free PSUM bank and applied by one DVE
op reading that PSUM operand.  LayerNorm is done feature-major: column sums
via PE ones-matmuls, rstd = exp(-0.5*ln(v)) on ACT (single activation-table
set, zero table switches).  Residuals follow the reference post-LN chaining
(z = LN(y + y2), z_end = LN(z + z2), out = LN(z_end + ff)).

Programs are specialized at build time to the actual kv lengths (read from
the int32 length inputs), so masked kv tiles are skipped entirely; up to 8
distinct programs (4 batches x even/odd query half) are compiled and
launched concurrently on disjoint device subsets.
"""

import os
import sys
import threading

import numpy as np

for _p in ("/opt/trn_rl_repo", "/root/.axon_site"):
    if os.path.isdir(_p) and _p not in sys.path:
        sys.path.append(_p)

import ml_dtypes
from contextlib import ExitStack

import concourse.bass as bass
import concourse.mybir as mybir
from concourse import bacc
from concourse.tile import TileContext

BF16 = np.float16
F32 = np.float32
NEG = -1000000.0
B, T, S, TM, KW, D, H, DFF = 4, 1024, 1024, 512, 64, 512, 8, 2048
DH = D // H  # 64
P = 128
NCH = D // P  # 4 feature chunks
AF = mybir.ActivationFunctionType
OP = mybir.AluOpType


# ---------------------------------------------------------------------------
# program builder
# ---------------------------------------------------------------------------

def build_program(qh, kts_cc, kts_ct, gate_b=0.0, apply_affine=False, debug=False):
    """Build one core's Bass program.

    qh: 0/1 query half.  kts_cc/kts_ct: number of 128-wide kv tiles for the
    source-code / template cross attentions (specialized to actual length).
    """
    f32, bf16 = mybir.dt.float32, mybir.dt.float16
    KV = 512 * (qh + 1)          # self-attn kv range
    QOFF = qh * 512              # q columns inside xkvT

    nc = bacc.Bacc("TRN2", target_bir_lowering=False, debug=False)

    def din(name, shape, dt=bf16):
        return nc.dram_tensor(name, shape, dt, kind="ExternalInput").ap()

    fp8 = mybir.dt.float8e4
    xkv8T = din("xkv8T", [D, KV], fp8)
    src8T = din("src8T", [D, kts_cc * P], fp8)
    tmpl8T = din("tmpl8T", [D, kts_ct * P], fp8)
    kw8T = din("kw8T", [D, KW], fp8)
    xqT = din("xqT", [D, 512])
    W8NAMES = ("sa_wk", "sa_wv", "sa_wq", "cc_wk", "cc_wv",
               "ck_wk", "ck_wv", "ct_wk", "ct_wv")
    wnames = [f"{n}_{p}" for n in ("sa", "cc", "ct", "ck")
              for p in ("wq", "wk", "wv", "wo")]
    wd = {n: din(n, [D, D], fp8 if n in W8NAMES else bf16)
          for n in wnames}
    w1d = din("ffn_w1", [D, DFF])
    w2d = din("ffn_w2", [DFF, D])
    gwA = din("gwA", [D, 1])
    gwB = din("gwB", [D, 1])
    staird = din("stair", [P, P])
    ccbias_d = din("cc_bias", [P, 1], f32)
    ctbias_d = din("ct_bias", [P, 1], f32)
    kwbias_d = din("kw_bias", [KW, 1], f32)
    affine_d = din("ln_affine", [P, NCH * 8], f32) if apply_affine else None
    outT = nc.dram_tensor("outT", [D, 512], bf16, kind="ExternalOutput").ap()
    dbg_outs = {}

    def mkdbg(nm, shape):
        if nm not in dbg_outs:
            dbg_outs[nm] = nc.dram_tensor(f"dbg_{nm}", shape, f32,
                                          kind="ExternalOutput").ap()
        return dbg_outs[nm]

    with TileContext(nc, pool_alloc_mode="queue") as tc, ExitStack() as ctx:
        # Pin the activation table to natural_log_exp_and_others (set 6):
        # it contains exp/ln/square/relu/copy/identity, i.e. every ACT
        # function this program uses, so no further table loads are needed.
        nc.scalar.add_instruction(mybir.InstLoadActFuncSet(
            name=nc.get_next_instruction_name(), act_func_set_id=6,
            ins=[], outs=[]))
        pers = ctx.enter_context(tc.tile_pool(name="pers", bufs=1))
        # ---- persistent small constants -------------------------------
        stair = pers.tile([P, P], bf16, name="stair_t")
        nc.sync.dma_start(out=stair, in_=staird)
        # selector for the head-pair 1/n broadcast: row 0 -> partitions
        # 0:64 (even head), row 32 -> partitions 64:128 (odd head)
        # head-pair 1/n machinery: denominators for pairs (0,1) land on
        # rows 0/32/64/96 of tile A, pairs (2,3) on tile B (32-aligned
        # partition writes only).  One DVE reciprocal+cast per tile.
        # selAB col block hp%2 maps rows (0,32) or (64,96) onto the
        # 64-partition halves of a pair's output.
        selAB = pers.tile([97, 2 * P], bf16, name="selAB_t")
        nc.vector.memset(selAB, 0.0)
        nc.gpsimd.memset(selAB[0:1, 0:DH], 1.0)
        nc.gpsimd.memset(selAB[32:33, DH:P], 1.0)
        nc.gpsimd.memset(selAB[64:65, P:P + DH], 1.0)
        nc.gpsimd.memset(selAB[96:97, P + DH:2 * P], 1.0)
        nden97 = [pers.tile([97, 512], f32, name=f"nden97_{i}")
                  for i in range(2)]
        for t in nden97:
            nc.vector.memset(t, 1.0)
        nrec97 = [pers.tile([97, 512], f32, name=f"nrec97_{i}")
                  for i in range(2)]
        ninv97 = [pers.tile([97, 512], bf16, name=f"ninv97_{i}")
                  for i in range(2)]
        ccbias = pers.tile([P, 1], f32, name="ccbias_t")
        nc.sync.dma_start(out=ccbias, in_=ccbias_d)
        ctbias = pers.tile([P, 1], f32, name="ctbias_t")
        nc.sync.dma_start(out=ctbias, in_=ctbias_d)
        kwbias = pers.tile([KW, 1], f32, name="kwbias_t")
        nc.sync.dma_start(out=kwbias, in_=kwbias_d)
        gwa_t = pers.tile([P, NCH], bf16, name="gwa_t")
        nc.sync.dma_start(out=gwa_t, in_=gwA.rearrange("(i p) o -> p i o", p=P))
        gwb_t = pers.tile([P, NCH], bf16, name="gwb_t")
        nc.sync.dma_start(out=gwb_t, in_=gwB.rearrange("(i p) o -> p i o", p=P))
        ones_b = pers.tile([P, 1], bf16, name="ones_b")
        nc.vector.memset(ones_b, 1.0)
        ones_row = pers.tile([1, P], bf16, name="ones_row")
        nc.vector.memset(ones_row, 1.0)
        eps_t = pers.tile([1, 1], f32, name="eps_t")
        nc.vector.memset(eps_t, 1e-5)
        gb_t = pers.tile([1, 1], f32, name="gb_t")
        nc.vector.memset(gb_t, -float(gate_b))
        affine = None
        if apply_affine:
            affine = pers.tile([P, NCH * 8], f32, name="affine_t")
            nc.sync.dma_start(out=affine, in_=affine_d)

        def tap(nm, tiles):
            if not debug:
                return
            cols = tiles[0].shape[-1]
            d = mkdbg(nm, [len(tiles) * P, cols])
            for i, t in enumerate(tiles):
                rows = t.shape[0]
                nc.gpsimd.dma_start(out=d[i * P:i * P + rows, :], in_=t)

        # ---- global shared pools --------------------------------------
        # residual/LN-out tiles, reused across stages via shared tags
        rpool = ctx.enter_context(tc.tile_pool(name="rpool", bufs=1))

        def mktiles(nm, cols=512, dt=f32, n=NCH, tagp=None):
            tagp = tagp or nm
            return [rpool.tile([P, cols], dt, name=f"{nm}{i}", tag=f"{tagp}{i}",
                               bufs=1) for i in range(n)]

        # small 1/8-partition tiles + broadcast tiles, shared by all stages
        smallp = ctx.enter_context(tc.tile_pool(name="smallp", bufs=1))
        # transient [128, *] tiles (exp outputs, LN scratch, gate scratch)
        trp = ctx.enter_context(tc.tile_pool(name="trp", bufs=1))
        # PSUM: pps = projection/V accumulators; x_ps = paired scores
        # (2 banks each); x_po = AV out + LN stats + gate
        psA = ctx.enter_context(tc.tile_pool(name="psA", bufs=2, space="PSUM"))
        psB = ctx.enter_context(tc.tile_pool(name="psB", bufs=2, space="PSUM"))

        def load_w(pool, names):
            for n in names:
                if n in W8NAMES:
                    fp8 = mybir.dt.float8e4
                    wt[n] = pool.tile([P, 4 * D], fp8, name=f"{n}_t",
                                      tag=f"{n}_t", bufs=1)
                    nc.sync.dma_start(
                        out=wt[n].rearrange("p (c o n) -> p c o n",
                                            c=2, o=2),
                        in_=wd[n].rearrange("(c o p) n -> p c o n",
                                            c=2, o=2, p=P))
                else:
                    wt[n] = pool.tile([P, NCH * D], bf16, name=f"{n}_t",
                                      tag=f"{n}_t", bufs=1)
                    nc.sync.dma_start(
                        out=wt[n].rearrange("p (i n) -> p i n", n=D),
                        in_=wd[n].rearrange("(i p) n -> p i n", p=P))
        wt = {}

        def w8_lhsT(n, c, j):
            return wt[n].rearrange("p (c o n) -> p c o n",
                                   c=2, o=2)[:, c, :, j * P:(j + 1) * P]

        def w8_rhs(n, c):
            return wt[n].rearrange("p (c o n) -> p c o n", c=2, o=2)[:, c]

        def w8_flat(n, c, o, j):
            return wt[n].rearrange("p (c o n) -> p c o n",
                                   c=2, o=2)[:, c, o, j * P:(j + 1) * P]

        def load_act8(pool, nm, dram_ap, cols):
            fp8 = mybir.dt.float8e4
            tiles = []
            for c in range(2):
                t = pool.tile([P, 2 * cols], fp8, name=f"{nm}{c}",
                              tag=f"{nm}{c}", bufs=1)
                nc.sync.dma_start(
                    out=t.rearrange("p (o n) -> p o n", o=2),
                    in_=dram_ap[c * 2 * P:(c + 1) * 2 * P, :].rearrange(
                        "(o p) n -> p o n", o=2, p=P))
                tiles.append(t.rearrange("p (o n) -> p o n", o=2))
            return tiles

        def w_lhsT(n, i, j):
            return wt[n][:, i * D + j * P: i * D + (j + 1) * P]

        def w_rhs(n, i, cols=D):
            return wt[n][:, i * D: i * D + cols]

        def load_act(pool, nm, dram_ap, cols):
            tiles = []
            for i in range(NCH):
                t = pool.tile([P, cols], bf16, name=f"{nm}{i}",
                              tag=f"{nm}{i}", bufs=1)
                nc.sync.dma_start(out=t, in_=dram_ap[i * P:(i + 1) * P, :])
                tiles.append(t)
            return tiles

        # ----------------------------------------------------------------
        # helpers
        # ----------------------------------------------------------------
        def proj_fm_groups(wn, rhs_tiles, ncols, out_tiles, evict):
            """Per-psum-group closures for a feature-major projection; each
            emits 4 accumulating matmuls + one eviction."""
            ntt = (ncols + 511) // 512
            groups = []
            for j in range(NCH):
                for t in range(ntt):
                    def g(j=j, t=t):
                        cs = t * 512
                        ce = min(ncols, cs + 512)
                        ps = psA.tile([P, ce - cs], mybir.dt.float32,
                                      name="proj_ps", tag="pps")
                        for i in range(NCH):
                            nc.tensor.matmul(ps, w_lhsT(wn, i, j),
                                             rhs_tiles[i][:, cs:ce],
                                             start=(i == 0),
                                             stop=(i == NCH - 1))
                        evict(j, cs, ce, ps, out_tiles)
                    groups.append(g)
            return groups

        def proj_fm(wn, rhs_tiles, ncols, out_tiles, evict):
            for g in proj_fm_groups(wn, rhs_tiles, ncols, out_tiles, evict):
                g()

        DR = mybir.MatmulPerfMode.DoubleRow
        DESC = 1.0 / 1024.0  # descale: activation x16, weight x64

        def evict_ds(j, cs, ce, ps, out_tiles, balance=False):
            if balance and (j + cs // 512) % 2 == 1:
                nc.scalar.mul(out_tiles[j][:, cs:ce], ps, DESC)
            else:
                nc.vector.tensor_scalar_mul(out_tiles[j][:, cs:ce], ps, DESC)

        def proj_fm8_groups(wn, rhs8, ncols, out_tiles, balance=False):
            """fp8 DoubleRow feature-major projection (descaled evict).
            Falls back to normal-mode fp8 matmuls when ncols < 128."""
            ntt = (ncols + 511) // 512
            groups = []
            for j in range(NCH):
                for t in range(ntt):
                    def g(j=j, t=t):
                        cs = t * 512
                        ce = min(ncols, cs + 512)
                        ps = psA.tile([P, ce - cs], mybir.dt.float32,
                                      name="proj_ps", tag="pps")
                        if ncols >= P:
                            for c in range(2):
                                nc.tensor.matmul(
                                    ps, w8_lhsT(wn, c, j),
                                    rhs8[c][:, :, cs:ce],
                                    start=(c == 0), stop=(c == 1),
                                    perf_mode=DR)
                        else:
                            for ci in range(4):
                                c, o = ci // 2, ci % 2
                                nc.tensor.matmul(
                                    ps, w8_flat(wn, c, o, j),
                                    rhs8[c][:, o, cs:ce],
                                    start=(ci == 0), stop=(ci == 3))
                        evict_ds(j, cs, ce, ps, out_tiles, balance)
                    groups.append(g)
            return groups

        def proj_v8_groups(enc8, wn, nkv, vt_list, vpool, ktag):
            nch_tok = (nkv + P - 1) // P
            vt_list.extend(
                vpool.tile([min(P, nkv - m * P), H * (DH + 1)], bf16,
                           name=f"{ktag}_v{m}", tag=f"{ktag}_v{m}", bufs=1)
                for m in range(nch_tok))
            groups = []
            for m in range(nch_tok):
                def g(m=m):
                    rows = vt_list[m].shape[0]
                    ps = psA.tile([rows, D], mybir.dt.float32,
                                  name="v_ps", tag="pps")
                    for c in range(2):
                        nc.tensor.matmul(ps,
                                         enc8[c][:, :, m * P:m * P + rows],
                                         w8_rhs(wn, c),
                                         start=(c == 0), stop=(c == 1),
                                         perf_mode=DR)
                    vt = vt_list[m]
                    src3 = ps.rearrange("p (g c) -> p g c", c=DH)
                    dst3 = vt.rearrange("p (g c) -> p g c", c=DH + 1)
                    nc.vector.tensor_scalar_mul(dst3[:, :, 0:DH], src3, DESC)
                    nc.gpsimd.memset(dst3[:, :, DH:DH + 1], 1.0)
                groups.append(g)
            return groups

        def evict_copy(j, cs, ce, ps, out_tiles):
            nc.vector.tensor_copy(out_tiles[j][:, cs:ce], ps)

        def evict_copy_bal(j, cs, ce, ps, out_tiles):
            if (j + cs // 512) % 2 == 0:
                nc.vector.tensor_copy(out_tiles[j][:, cs:ce], ps)
            else:
                nc.scalar.copy(out_tiles[j][:, cs:ce], ps)

        def proj_v_groups(enc_tiles, wn, nkv, vt_list, vpool, ktag):
            nch_tok = (nkv + P - 1) // P
            vt_list.extend(
                vpool.tile([min(P, nkv - m * P), H * (DH + 1)], bf16,
                           name=f"{ktag}_v{m}", tag=f"{ktag}_v{m}", bufs=1)
                for m in range(nch_tok))
            groups = []
            for m in range(nch_tok):
                def g(m=m):
                    rows = vt_list[m].shape[0]
                    ps = psA.tile([rows, D], mybir.dt.float32,
                                  name="v_ps", tag="pps")
                    for i in range(NCH):
                        nc.tensor.matmul(ps,
                                         enc_tiles[i][:, m * P:m * P + rows],
                                         w_rhs(wn, i),
                                         start=(i == 0), stop=(i == NCH - 1))
                    vt = vt_list[m]
                    src3 = ps.rearrange("p (g c) -> p g c", c=DH)
                    dst3 = vt.rearrange("p (g c) -> p g c", c=DH + 1)
                    nc.vector.tensor_copy(dst3[:, :, 0:DH], src3)
                    nc.gpsimd.memset(dst3[:, :, DH:DH + 1], 1.0)
                groups.append(g)
            return groups

        def proj_v(enc_tiles, wn, nkv, vt_list, vpool, ktag):
            for g in proj_v_groups(enc_tiles, wn, nkv, vt_list, vpool, ktag):
                g()

        def attention(qt, kt, vt_list, out_tiles, bias_tile, causal, ktag,
                      fillers=None):
            """Multi-head attention.  Head pairs share one [rows,1024]
            scores psum + one merged exp; the AV matmul for tile kt is
            emitted after the scores matmul for tile kt+1 so the ACT exp
            overlaps PE work.  Causal scores/AV are restricted to the
            unmasked column range.  The softmax normalization runs per
            head-pair (collect denominators on partitions 0/32, one DVE
            reciprocal, one K=33 selector broadcast, one fused multiply)
            so it hides under the next pair's kt loop.  `fillers` is a
            list of closures emitting independent PE work; one is popped
            after each kt iteration to fill the exp-wait bubbles."""
            nkt = len(vt_list)
            fillers = fillers if fillers is not None else []

            def pop_filler():
                if fillers:
                    fillers.pop(0)()

            for hp in range(H // 2):
                po = []
                for s in range(2):
                    po.append(psB.tile([DH + 1, 512], mybir.dt.float32,
                                       name=f"{ktag}_po{s}", tag="x_po"))
                pend = None  # deferred AV: (kt_i, pt2, c0)

                def flush_av(last):
                    kt_i, pt2, c0 = pend
                    for s in range(2):
                        h = 2 * hp + s
                        nc.tensor.matmul(
                            po[s][:, c0:512],
                            vt_list[kt_i][:, h * (DH + 1):
                                          (h + 1) * (DH + 1)],
                            pt2[:, s * 512 + c0:(s + 1) * 512],
                            start=(kt_i == 0), stop=last)

                for kt_i in range(nkt):
                    rows = vt_list[kt_i].shape[0]
                    d = kt_i - (nkt - 4) if causal else -1
                    c0 = d * P if (causal and d > 0) else 0
                    ps2 = psB.tile([rows, 1024], mybir.dt.float32,
                                   name=f"{ktag}_ps", tag="x_ps")
                    pt2 = trp.tile([rows, 1024], bf16,
                                   name=f"{ktag}_pt", tag="pt", bufs=3)
                    for s in range(2):
                        ro = s * DH
                        o = s * 512
                        nc.tensor.matmul(
                            ps2[:, o + c0:o + 512],
                            kt[hp][ro:ro + DH, kt_i * P:kt_i * P + rows],
                            qt[hp][ro:ro + DH, c0:512], start=True, stop=True)
                    if causal and d >= 0:
                        for s in range(2):
                            o = s * 512
                            nc.vector.tensor_add(
                                ps2[:, o + d * P:o + (d + 1) * P],
                                ps2[:, o + d * P:o + (d + 1) * P], stair)
                            nc.scalar.activation(pt2[:, o + c0:o + 512],
                                                 ps2[:, o + c0:o + 512],
                                                 AF.Exp, scale=0.125)
                    else:
                        bias = 0.0
                        if bias_tile is not None and kt_i == nkt - 1:
                            bias = bias_tile[:rows, :]
                        nc.scalar.activation(pt2, ps2, AF.Exp,
                                             bias=bias, scale=0.125)
                    if pend is not None:
                        flush_av(False)
                        if kt_i % 2 == 1:
                            pop_filler()
                    pend = (kt_i, pt2, c0)
                flush_av(True)
                # stage this pair's denominators into tile A (pairs 0,1)
                # or B (pairs 2,3) on 32-aligned partitions
                ab, r0 = hp // 2, (hp % 2) * DH
                nc.vector.tensor_copy(nden97[ab][r0:r0 + 1, :],
                                      po[0][DH:DH + 1, :])
                nc.vector.tensor_copy(nden97[ab][r0 + 32:r0 + 33, :],
                                      po[1][DH:DH + 1, :])
                nc.scalar.copy(out_tiles[hp][0:DH, :], po[0][0:DH, :])
                nc.vector.tensor_copy(out_tiles[hp][DH:P, :], po[1][0:DH, :])
                if hp % 2 == 1:
                    # both pairs of this tile staged: one reciprocal+cast,
                    # then normalize both pairs (overlaps the next loop)
                    nc.vector.reciprocal_approx_fast(out=nrec97[ab],
                                                     in_=nden97[ab])
                    nc.vector.tensor_copy(ninv97[ab], nrec97[ab])
                    for hq in (hp - 1, hp):
                        pop_filler()
                        nb = psA.tile([P, 512], mybir.dt.float32,
                                      name=f"{ktag}_nb{hq}", tag="pps")
                        nc.tensor.matmul(nb,
                                         selAB[:, (hq % 2) * P:
                                               (hq % 2 + 1) * P],
                                         ninv97[ab], start=True, stop=True)
                        nc.vector.tensor_mul(out_tiles[hq], out_tiles[hq],
                                             nb)
            for g in fillers:
                g()

        def layernorm(r_tiles, out_tiles, ln_idx, mid=None):
            sq = [trp.tile([P, 512], bf16, name=f"ln{ln_idx}_sq", tag="ln_sq",
                           bufs=2) for _ in range(NCH)]
            for j in range(NCH):
                nc.scalar.activation(sq[j], r_tiles[j], AF.Square)
            ps_s = psB.tile([1, 512], mybir.dt.float32,
                            name="ln_ps_s", tag="x_po")
            ps_q = psB.tile([1, 512], mybir.dt.float32,
                            name="ln_ps_q", tag="x_po")
            for j in range(NCH):
                nc.tensor.matmul(ps_s, ones_b, r_tiles[j],
                                 start=(j == 0), stop=(j == NCH - 1))
            for j in range(NCH):
                nc.tensor.matmul(ps_q, ones_b, sq[j],
                                 start=(j == 0), stop=(j == NCH - 1))
            if mid is not None:
                mid()
            mean16 = smallp.tile([1, 512], bf16,
                                 name="ln_mean16", tag="ln_stat", bufs=3)
            nc.vector.tensor_scalar_mul(mean16, ps_s, 1.0 / D)
            meanb = psB.tile([P, 512], mybir.dt.float32,
                             name="ln_meanb", tag="x_po")
            nc.tensor.matmul(meanb, ones_row, mean16, start=True, stop=True)
            msq = smallp.tile([1, 512], mybir.dt.float32,
                              name="ln_msq", tag="ln_stat", bufs=3)
            nc.scalar.activation(msq, ps_s, AF.Square, scale=1.0 / D)
            var = smallp.tile([1, 512], mybir.dt.float32,
                              name="ln_var", tag="ln_stat", bufs=3)
            nc.vector.scalar_tensor_tensor(var, ps_q, 1.0 / D, msq,
                                           op0=OP.mult, op1=OP.subtract)
            lnv = smallp.tile([1, 512], mybir.dt.float32,
                              name="ln_lnv", tag="ln_stat", bufs=3)
            nc.scalar.activation(lnv, var, AF.Ln, bias=eps_t[:, :])
            rstd = smallp.tile([1, 512], bf16,
                               name="ln_rstd", tag="ln_stat", bufs=3)
            nc.scalar.activation(rstd, lnv, AF.Exp, scale=-0.5)
            rstdb = psB.tile([P, 512], mybir.dt.float32,
                             name="ln_rstdb", tag="x_po")
            nc.tensor.matmul(rstdb, ones_row, rstd, start=True, stop=True)
            for j in range(NCH):
                tmp = trp.tile([P, 512], bf16,
                               name="ln_tmp", tag="ln_tmp", bufs=2)
                nc.vector.tensor_sub(tmp, r_tiles[j], meanb)
                nc.vector.tensor_mul(out_tiles[j], tmp, rstdb)
                if apply_affine:
                    g = affine[:, ln_idx * 2 * NCH + j: ln_idx * 2 * NCH + j + 1]
                    b = affine[:, ln_idx * 2 * NCH + NCH + j:
                               ln_idx * 2 * NCH + NCH + j + 1]
                    nc.vector.tensor_scalar(out_tiles[j], out_tiles[j],
                                            g, b, op0=OP.mult, op1=OP.add)

        # ================================================================
        # emission (ordered for cross-stage overlap)
        # ================================================================
        r1 = mktiles("r1", dt=bf16, tagp="rA")
        y = mktiles("y", dt=bf16, tagp="lnA")
        r2 = mktiles("r2", dt=bf16, tagp="rB")
        z = mktiles("z", dt=bf16, tagp="lnB")
        r3 = None  # allocated after r1 dies
        ze = None

        # ct pool created first so it outlives ccsb (LIFO pool stack);
        # its DMA loads are issued after LN1 and overlap the cc/ck stage
        ctsb = ctx.enter_context(tc.tile_pool(name="tail_sb", bufs=1))
        ccsb_cm = tc.tile_pool(name="cc_sb", bufs=1)
        ccsb = ccsb_cm.__enter__()
        sasb_cm = tc.tile_pool(name="sa_sb", bufs=1)
        sasb = sasb_cm.__enter__()

        # --- stage 1: self attention (fp8 DoubleRow K/V/Q) ---
        load_w(sasb, ["sa_wk"])
        xkv8 = load_act8(sasb, "xkv8", xkv8T, KV)
        xq = load_act(sasb, "xq", xqT, 512)
        load_w(sasb, ["sa_wv", "sa_wq", "sa_wo"])
        qt = [sasb.tile([P, 512], bf16, name=f"sa_q{i}", tag=f"sa_q{i}",
                        bufs=1) for i in range(NCH)]
        ktl = [sasb.tile([P, KV], bf16, name=f"sa_k{i}", tag=f"sa_k{i}",
                         bufs=1) for i in range(NCH)]
        xq8 = [t[:, :, QOFF:QOFF + 512] for t in xkv8]
        for g in proj_fm8_groups("sa_wk", xkv8, KV, ktl, balance=True):
            g()
        vts = []
        for g in proj_v8_groups(xkv8, "sa_wv", KV, vts, sasb, "sa"):
            g()
        for g in proj_fm8_groups("sa_wq", xq8, 512, qt, balance=True):
            g()
        at = [trp.tile([P, 512], bf16, name=f"sa_at{i}", tag=f"at{i}",
                       bufs=1) for i in range(NCH)]
        # cc K/V projections are independent of sa: interleave them into
        # sa's kt loops as PE fillers (their DMA loads were issued above)
        load_w(ccsb, ["cc_wk", "cc_wv", "ck_wk", "ck_wv",
                      "cc_wq", "ck_wq", "cc_wo", "ck_wo"])
        srcl8 = load_act8(ccsb, "src8", src8T, kts_cc * P)
        kwe8 = load_act8(ccsb, "kw8", kw8T, KW)
        cc_kt = [ccsb.tile([P, kts_cc * P], bf16, name=f"cc_k{i}",
                           tag=f"cc_k{i}", bufs=1) for i in range(NCH)]
        cc_vts = []
        sa_fill = (proj_fm8_groups("cc_wk", srcl8, kts_cc * P, cc_kt)
                   + proj_v8_groups(srcl8, "cc_wv", kts_cc * P, cc_vts,
                                    ccsb, "cc"))
        attention(qt, ktl, vts, at, None, True, "sa", fillers=sa_fill)

        def evict_resid_x(j, cs, ce, ps, out_tiles):
            nc.vector.tensor_add(out_tiles[j][:, cs:ce], ps, xq[j])
        tap("sa_at", at)
        proj_fm("sa_wo", at, 512, r1, evict_resid_x)
        tap("r1", r1)
        ck_kt = [ccsb.tile([P, KW], bf16, name=f"ck_k{i}", tag=f"ck_k{i}",
                           bufs=1) for i in range(NCH)]
        ck_vts = []

        def ln1_mid():
            for g in proj_fm8_groups("ck_wk", kwe8, KW, ck_kt):
                g()
            for g in proj_v8_groups(kwe8, "ck_wv", KW, ck_vts, ccsb, "ck"):
                g()
        layernorm(r1, y, 0, mid=ln1_mid)
        tap("y", y)
        sasb_cm.__exit__(None, None, None)

        # ct weight/activation DMA loads overlap the whole cc/ck stage
        load_w(ctsb, ["ct_wk", "ct_wv", "ct_wq", "ct_wo"])
        tmpl8 = load_act8(ctsb, "tmpl8", tmpl8T, kts_ct * P)

        # --- stage 2: cc + ck cross attention + gate ---
        cc_qt = [ccsb.tile([P, 512], bf16, name=f"cc_q{i}", tag=f"cc_q{i}",
                           bufs=1) for i in range(NCH)]
        proj_fm("cc_wq", y, 512, cc_qt, evict_copy_bal)
        cc_at = [trp.tile([P, 512], bf16, name=f"cc_at{i}", tag=f"at{i}",
                          bufs=1) for i in range(NCH)]
        ck_qt = [ccsb.tile([P, 512], bf16, name=f"ck_q{i}", tag=f"ck_q{i}",
                           bufs=1) for i in range(NCH)]
        cc_fill = proj_fm_groups("ck_wq", y, 512, ck_qt, evict_copy)
        attention(cc_qt, cc_kt, cc_vts, cc_at, ccbias, False, "cc",
                  fillers=cc_fill)
        ck_at = [trp.tile([P, 512], bf16, name=f"ck_at{i}", tag=f"ckat{i}",
                          bufs=1) for i in range(NCH)]
        y2c = [ccsb.tile([P, 512], bf16, name=f"y2c{i}", tag=f"y2c{i}",
                         bufs=1) for i in range(NCH)]
        ck_fill = proj_fm_groups("cc_wo", cc_at, 512, y2c, evict_copy)
        attention(ck_qt, ck_kt, ck_vts, ck_at, kwbias, False, "ck",
                  fillers=ck_fill)
        y2k = [ccsb.tile([P, 512], bf16, name=f"y2k{i}", tag=f"y2k{i}",
                         bufs=1) for i in range(NCH)]
        proj_fm("ck_wo", ck_at, 512, y2k, evict_copy_bal)

        # --- gate ---
        ps_g = psB.tile([1, 512], mybir.dt.float32, name="gate_ps",
                        tag="x_po")
        for i in range(NCH):
            nc.tensor.matmul(ps_g, gwa_t[:, i:i + 1], y2c[i],
                             start=(i == 0), stop=False)
        for i in range(NCH):
            nc.tensor.matmul(ps_g, gwb_t[:, i:i + 1], y2k[i],
                             start=False, stop=(i == NCH - 1))
        # g-independent combine pieces, overlap the gate ACT/DVE chain
        gdt = [trp.tile([P, 512], bf16, name=f"gate_dt{j}", tag=f"gate_dt{j}",
                        bufs=1) for j in range(NCH)]
        for j in range(NCH):
            nc.vector.tensor_sub(gdt[j], y2c[j], y2k[j])
            nc.vector.tensor_add(r2[j], y[j], y2k[j])
        ct_kt = [ctsb.tile([P, kts_ct * P], bf16, name=f"ct_k{i}",
                           tag=f"ct_k{i}", bufs=1) for i in range(NCH)]

        def ct_mid():
            for g in proj_fm8_groups("ct_wk", tmpl8, kts_ct * P, ct_kt):
                g()
        ge = smallp.tile([1, 512], mybir.dt.float32, name="gate_e",
                         tag="gate_edg", bufs=2)
        nc.scalar.activation(ge, ps_g, AF.Exp, scale=-1.0, bias=gb_t[:, :])
        gp1 = smallp.tile([1, 512], mybir.dt.float32, name="gate_p1",
                          tag="gate_edg", bufs=2)
        nc.vector.tensor_scalar_add(gp1, ge, 1.0)
        grc = smallp.tile([1, 512], mybir.dt.float32, name="gate_rc",
                          tag="gate_edg", bufs=2)
        nc.vector.reciprocal_approx_fast(out=grc, in_=gp1)
        gg = smallp.tile([1, 512], bf16, name="gate_g",
                         tag="gate_edg", bufs=2)
        nc.vector.tensor_copy(gg, grc)
        ct_mid()
        ggb = psB.tile([P, 512], mybir.dt.float32, name="gate_gb",
                       tag="x_po")
        nc.tensor.matmul(ggb, ones_row, gg, start=True, stop=True)
        # r2 = (y + y2k) + g*(y2c - y2k)
        for j in range(NCH):
            nc.vector.tensor_mul(gdt[j], gdt[j], ggb)
            nc.vector.tensor_add(r2[j], r2[j], gdt[j])
        tap("y2c", y2c)
        tap("y2k", y2k)
        tap("r2", r2)
        ccsb_cm.__exit__(None, None, None)
        # FFN weights: DMA overlaps the ct attention stage
        ffsb = ctx.enter_context(tc.tile_pool(name="ff_sb", bufs=1))
        w1t = ffsb.tile([P, NCH * DFF], bf16, name="w1_t", tag="w1_t")
        nc.sync.dma_start(out=w1t.rearrange("p (i n) -> p i n", n=DFF),
                          in_=w1d.rearrange("(i p) n -> p i n", p=P))
        w2t = ffsb.tile([P, (DFF // P) * D], bf16, name="w2_t", tag="w2_t")
        nc.sync.dma_start(out=w2t.rearrange("p (i n) -> p i n", n=D),
                          in_=w2d.rearrange("(i p) n -> p i n", p=P))
        ct_vts = []

        def ln2_mid():
            for g in proj_v8_groups(tmpl8, "ct_wv", kts_ct * P, ct_vts,
                                    ctsb, "ct"):
                g()
        layernorm(r2, z, 1, mid=ln2_mid)
        tap("z", z)

        # --- stage 3: ct cross attention ---
        r3 = mktiles("r3", dt=bf16, tagp="rA")
        ze = mktiles("ze", dt=bf16, tagp="lnA")
        ct_qt = [ffsb.tile([P, 512], bf16, name=f"ct_q{i}", tag=f"ct_q{i}",
                           bufs=1) for i in range(NCH)]
        proj_fm("ct_wq", z, 512, ct_qt, evict_copy_bal)
        ct_at = [trp.tile([P, 512], bf16, name=f"ct_at{i}", tag=f"at{i}",
                          bufs=1) for i in range(NCH)]
        attention(ct_qt, ct_kt, ct_vts, ct_at, ctbias, False, "ct")

        def evict_resid_r2(j, cs, ce, ps, out_tiles):
            nc.vector.tensor_add(out_tiles[j][:, cs:ce], ps, z[j])
        tap("ct_at", ct_at)
        proj_fm("ct_wo", ct_at, 512, r3, evict_resid_r2)
        tap("r3", r3)
        layernorm(r3, ze, 2)
        tap("ze", ze)

        # --- stage 4: FFN ---
        ht = [ffsb.tile([P, 512], bf16, name=f"ff_h{i}", tag=f"ff_h{i}",
                        bufs=1) for i in range(DFF // P)]
        for jf in range(DFF // P):
            ps = psA.tile([P, 512], mybir.dt.float32, name="ff_ps",
                          tag="pps")
            for i in range(NCH):
                nc.tensor.matmul(ps, w1t[:, i * DFF + jf * P:
                                         i * DFF + (jf + 1) * P],
                                 ze[i], start=(i == 0), stop=(i == NCH - 1))
            if jf % 2 == 0:
                nc.scalar.activation(ht[jf], ps, AF.Relu)
            else:
                nc.vector.tensor_scalar_max(ht[jf], ps, 0.0)
        r4 = mktiles("r4", dt=bf16, tagp="rB")
        for j in range(NCH):
            ps = psA.tile([P, 512], mybir.dt.float32, name="ff_ps2",
                          tag="pps")
            for i in range(DFF // P):
                nc.tensor.matmul(ps, w2t[:, i * D + j * P: i * D + (j + 1) * P],
                                 ht[i], start=(i == 0),
                                 stop=(i == DFF // P - 1))
            nc.vector.tensor_add(r4[j], ps, ze[j])
        fin = [trp.tile([P, 512], bf16, name=f"fin{i}",
                        tag=f"at{i}", bufs=1) for i in range(NCH)]
        layernorm(r4, fin, 3)
        for j in range(NCH):
            nc.sync.dma_start(out=outT[j * P:(j + 1) * P, :], in_=fin[j])

    nc.compile()
    return nc


# ---------------------------------------------------------------------------
# host-side input preparation
# ---------------------------------------------------------------------------

W8NAMES_H = ("sa_wk", "sa_wv", "sa_wq", "cc_wk", "cc_wv",
             "ck_wk", "ck_wv", "ct_wk", "ct_wv")
FP8 = ml_dtypes.float8_e4m3
SW8, SX8 = 64.0, 16.0


def _prep_shared(inputs):
    """Cast/transform weights shared by every core."""
    sh = {}
    for n in ("sa", "cc", "ct", "ck"):
        for p in ("wq", "wk", "wv", "wo"):
            nm = f"{n}_{p}"
            if nm in W8NAMES_H:
                sh[nm] = np.ascontiguousarray(
                    np.clip(inputs[nm].astype(F32) * SW8,
                            -240, 240).astype(FP8))
            else:
                sh[nm] = np.ascontiguousarray(inputs[nm].astype(BF16))
    sh["ffn_w1"] = np.ascontiguousarray(inputs["ffn_w1"].astype(BF16))
    sh["ffn_w2"] = np.ascontiguousarray(inputs["ffn_w2"].astype(BF16))
    gw = inputs["gate_w"].astype(F32)
    sh["gwA"] = np.ascontiguousarray(gw[:D].astype(BF16))
    sh["gwB"] = np.ascontiguousarray(gw[D:].astype(BF16))
    kl, ql = np.arange(P)[:, None], np.arange(P)[None, :]
    sh["stair"] = np.where(kl <= ql, 0.0, NEG).astype(BF16)
    return sh


def _len_bias(L, kts, width=P):
    """[width,1] f32 additive bias for the LAST kv tile."""
    base = (kts - 1) * P
    idx = base + np.arange(width)
    return np.where(idx < L, 0.0, NEG).astype(F32)[:, None]


def _q8(a):
    return np.clip(a.astype(F32) * SX8, -240, 240).astype(FP8)


def _prep_core(inputs, sh, b, qh, kts_cc, kts_ct):
    KVn = 512 * (qh + 1)
    m = dict(sh)
    xT = inputs["x"][b].T.astype(F32)  # [D, T]
    m["xkv8T"] = np.ascontiguousarray(_q8(xT[:, :KVn]))
    m["xqT"] = np.ascontiguousarray(
        xT[:, qh * 512:(qh + 1) * 512].astype(BF16))
    Ls = int(inputs["source_code_len"][b])
    st = np.zeros((D, kts_cc * P), FP8)
    st[:, :Ls] = _q8(inputs["source_code_enc"][b, :Ls].T)
    m["src8T"] = st
    Lt = int(inputs["template_len"][b])
    tt = np.zeros((D, kts_ct * P), FP8)
    tt[:, :Lt] = _q8(inputs["template_enc"][b, :Lt].T)
    m["tmpl8T"] = tt
    m["kw8T"] = np.ascontiguousarray(_q8(inputs["keywords_enc"][b].T))
    m["cc_bias"] = _len_bias(Ls, kts_cc)
    m["ct_bias"] = _len_bias(Lt, kts_ct)
    m["kw_bias"] = _len_bias(int(inputs["keywords_len"][b]), 1, KW)
    return m


# ---------------------------------------------------------------------------
# concurrent multi-program PJRT runner (adapted from bass2jax.run_bass_via_pjrt)
# ---------------------------------------------------------------------------

def _run_groups(groups):
    """groups: list of (nc, core_ids, in_maps).  Dispatch all groups onto
    their own device subsets, then gather.  Returns {core_id: {name: arr}}."""
    import jax
    import numpy as _np
    from jax.sharding import Mesh, PartitionSpec
    from jax.experimental.shard_map import shard_map
    from concourse import bass2jax
    from concourse.bass2jax import (_bass_exec_p, install_neuronx_cc_hook,
                                    partition_id_tensor)

    install_neuronx_cc_hook()
    devices = jax.devices()

    def make_launch(nc, core_ids, in_maps):
        pname = (nc.partition_id_tensor.name
                 if nc.partition_id_tensor else None)
        in_names, out_names, out_avals, zero_outs = [], [], [], []
        for alloc in nc.m.functions[0].allocations:
            if not isinstance(alloc, mybir.MemoryLocationSet):
                continue
            name = alloc.memorylocations[0].name
            if alloc.kind == "ExternalInput":
                if name == pname:
                    continue
                in_names.append(name)
            elif alloc.kind == "ExternalOutput":
                shape = tuple(alloc.tensor_shape)
                dtype = mybir.dt.np(alloc.dtype)
                out_names.append(name)
                out_avals.append(jax.core.ShapedArray(shape, dtype))
                zero_outs.append(_np.zeros(shape, dtype))
        n_params, n_outs = len(in_names), len(out_avals)
        all_in_names = in_names + out_names
        if pname is not None:
            all_in_names = all_in_names + [pname]

        def _body(*args):
            operands = list(args)
            if pname is not None:
                operands.append(partition_id_tensor())
            outs = _bass_exec_p.bind(
                *operands, out_avals=tuple(out_avals),
                in_names=tuple(all_in_names), out_names=tuple(out_names),
                lowering_input_output_aliases=(),
                sim_require_finite=False, sim_require_nnan=False, nc=nc)
            return tuple(outs)

        donate = tuple(range(n_params, n_params + n_outs))
        devs = [devices[c] for c in core_ids]
        if len(core_ids) == 1:
            fn = jax.jit(_body, donate_argnums=donate, keep_unused=True,
                         device=devs[0])
            args = [in_maps[0][nm] for nm in in_names] + list(zero_outs)
            out_arrs = fn(*args)
            return out_names, out_avals, out_arrs, None
        mesh = Mesh(_np.asarray(devs), ("core",))
        in_specs = (PartitionSpec("core"),) * (n_params + n_outs)
        out_specs = (PartitionSpec("core"),) * n_outs
        fn = jax.jit(shard_map(_body, mesh=mesh, in_specs=in_specs,
                               out_specs=out_specs, check_rep=False),
                     donate_argnums=donate, keep_unused=True)
        cat = [_np.concatenate([_np.asarray(m[nm]) for m in in_maps], axis=0)
               for nm in in_names]
        catz = [_np.zeros((len(core_ids) * z.shape[0], *z.shape[1:]), z.dtype)
                for z in zero_outs]
        out_arrs = fn(*cat, *catz)
        return out_names, out_avals, out_arrs, len(core_ids)

    last_err = None
    for _attempt in range(3):
        try:
            launched = []
            for nc, core_ids, in_maps in groups:
                launched.append((core_ids, make_launch(nc, core_ids, in_maps)))
            results = {}
            for core_ids, (out_names, out_avals, out_arrs, ncores) in launched:
                if ncores is None:
                    results[core_ids[0]] = {nm: _np.asarray(out_arrs[i])
                                            for i, nm in enumerate(out_names)}
                else:
                    for ci, c in enumerate(core_ids):
                        results[c] = {
                            nm: _np.asarray(out_arrs[i]).reshape(
                                ncores, *out_avals[i].shape)[ci]
                            for i, nm in enumerate(out_names)}
            return results
        except Exception as e:  # transient NRT device errors: retry
            last_err = e
            import time as _time
            _time.sleep(2.0)
    raise last_err


_PROGRAM_CACHE = {}
_CACHE_LOCK = threading.Lock()


def _get_program(key):
    with _CACHE_LOCK:
        if key in _PROGRAM_CACHE:
            return _PROGRAM_CACHE[key]
    qh, kts_cc, kts_ct, gate_b, aff = key
    nc = build_program(qh, kts_cc, kts_ct, gate_b=gate_b, apply_affine=aff)
    with _CACHE_LOCK:
        _PROGRAM_CACHE[key] = nc
    return nc


# ---------------------------------------------------------------------------
# entry point
# ---------------------------------------------------------------------------

def kernel(**inputs):
    inputs = {k: np.asarray(v) for k, v in inputs.items()}
    gate_b = float(inputs["gate_b"].reshape(-1)[0])
    aff = not all(
        np.all(inputs[f"ln{j}_g"] == 1.0) and np.all(inputs[f"ln{j}_b"] == 0.0)
        for j in range(1, 5))
    affine_arr = None
    if aff:
        affine_arr = np.zeros((P, NCH * 8), F32)
        for ln in range(4):
            g = inputs[f"ln{ln + 1}_g"].astype(F32).reshape(NCH, P).T
            bb = inputs[f"ln{ln + 1}_b"].astype(F32).reshape(NCH, P).T
            affine_arr[:, ln * 2 * NCH: ln * 2 * NCH + NCH] = g
            affine_arr[:, ln * 2 * NCH + NCH: (ln + 1) * 2 * NCH] = bb

    sh = _prep_shared(inputs)
    # core -> (program key, in_map)
    core_keys, core_maps = [], []
    for c in range(8):
        b, qh = c // 2, c % 2
        kts_cc = max(1, -(-int(inputs["source_code_len"][b]) // P))
        kts_ct = max(1, -(-int(inputs["template_len"][b]) // P))
        key = (qh, kts_cc, kts_ct, gate_b, aff)
        m = _prep_core(inputs, sh, b, qh, kts_cc, kts_ct)
        if aff:
            m["ln_affine"] = affine_arr
        core_keys.append(key)
        core_maps.append(m)

    # build distinct programs (parallel threads: walrus compile is subprocess)
    distinct = sorted(set(core_keys))
    threads = [threading.Thread(target=_get_program, args=(k,))
               for k in distinct]
    for t in threads:
        t.start()
    for t in threads:
        t.join()

    groups = []
    for key in distinct:
        cores = [c for c in range(8) if core_keys[c] == key]
        groups.append((_get_program(key), cores, [core_maps[c] for c in cores]))

    results = _run_groups(groups)

    out = np.empty((B, T, D), np.float32)
    for c in range(8):
        b, qh = c // 2, c % 2
        out[b, qh * 512:(qh + 1) * 512, :] = results[c]["outT"].T
    return out



# revision 40
# speedup vs baseline: 1.1237x; 1.0242x over previous
"""Trainium2 Bass kernel for nn_DecoderBlockWithKeywords.

Decoder block: causal self-attn + gated (source-code / keywords) cross-attn
+ template cross-attn + FFN, with 4 LayerNorms.  B=4, T=1024, D=512, H=8,
dh=64, DFF=2048.

Sharding: pure data-parallel over (batch, query-half) -> 8 NeuronCores, no
collectives.  Each core holds all weights (fp16) and computes 512 query
tokens of one batch element.

Layout strategy: every activation lives feature-major (X^T: [D on
partitions, tokens on free]).  Host pre-transposes/casts inputs.  Q/K
projections are weight-stationary (out feature-major); V is produced
token-major via activation-stationary matmuls so the attention AV matmul
needs no transposes at all.  Scores are computed as S^T = K_h Q_h^T
([kv, q]); softmax runs without max-subtraction (logits are O(1); masked
lanes get -1e6 bias fused into the ACT exp).  Softmax denominators come
from a ones-column appended to V inside the same AV matmul; per-column
scales (softmax 1/n, LN mean/rstd, gate g) are broadcast across partitions
with a PE ones-outer-product into a free PSUM bank and applied by one DVE
op reading that PSUM operand.  LayerNorm is done feature-major: column sums
via PE ones-matmuls, rstd = exp(-0.5*ln(v)) on ACT (single activation-table
set, zero table switches).  Residuals follow the reference post-LN chaining
(z = LN(y + y2), z_end = LN(z + z2), out = LN(z_end + ff)).

Programs are specialized at build time to the actual kv lengths (read from
the int32 length inputs), so masked kv tiles are skipped entirely; up to 8
distinct programs (4 batches x even/odd query half) are compiled and
launched concurrently on disjoint device subsets.
"""

import os
import sys
import threading

import numpy as np

for _p in ("/opt/trn_rl_repo", "/root/.axon_site"):
    if os.path.isdir(_p) and _p not in sys.path:
        sys.path.append(_p)

import ml_dtypes
from contextlib import ExitStack

import concourse.bass as bass
import concourse.mybir as mybir
from concourse import bacc
from concourse.tile import TileContext

BF16 = np.float16
F32 = np.float32
NEG = -1000000.0
B, T, S, TM, KW, D, H, DFF = 4, 1024, 1024, 512, 64, 512, 8, 2048
DH = D // H  # 64
P = 128
NCH = D // P  # 4 feature chunks
AF = mybir.ActivationFunctionType
OP = mybir.AluOpType


# ---------------------------------------------------------------------------
# program builder
# ---------------------------------------------------------------------------

def build_program(qh, kts_cc, kts_ct, gate_b=0.0, apply_affine=False, debug=False):
    """Build one core's Bass program.

    qh: 0/1 query half.  kts_cc/kts_ct: number of 128-wide kv tiles for the
    source-code / template cross attentions (specialized to actual length).
    """
    f32, bf16 = mybir.dt.float32, mybir.dt.float16
    KV = 512 * (qh + 1)          # self-attn kv range
    QOFF = qh * 512              # q columns inside xkvT

    nc = bacc.Bacc("TRN2", target_bir_lowering=False, debug=False)

    def din(name, shape, dt=bf16):
        return nc.dram_tensor(name, shape, dt, kind="ExternalInput").ap()

    fp8 = mybir.dt.float8e4
    xkv8T = din("xkv8T", [D, KV], fp8)
    src8T = din("src8T", [D, kts_cc * P], fp8)
    tmpl8T = din("tmpl8T", [D, kts_ct * P], fp8)
    kw8T = din("kw8T", [D, KW], fp8)
    xqT = din("xqT", [D, 512])
    W8NAMES = ("sa_wk", "sa_wv", "sa_wq", "cc_wk", "cc_wv",
               "ck_wk", "ck_wv", "ct_wk", "ct_wv")
    wnames = [f"{n}_{p}" for n in ("sa", "cc", "ct", "ck")
              for p in ("wq", "wk", "wv", "wo")]
    wd = {n: din(n, [D, D], fp8 if n in W8NAMES else bf16)
          for n in wnames}
    w1d = din("ffn_w1", [D, DFF])
    w2d = din("ffn_w2", [DFF, D])
    gwA = din("gwA", [D, 1])
    gwB = din("gwB", [D, 1])
    staird = din("stair", [P, P])
    ccbias_d = din("cc_bias", [P, 1], f32)
    ctbias_d = din("ct_bias", [P, 1], f32)
    kwbias_d = din("kw_bias", [KW, 1], f32)
    affine_d = din("ln_affine", [P, NCH * 8], f32) if apply_affine else None
    outT = nc.dram_tensor("outT", [D, 512], bf16, kind="ExternalOutput").ap()
    dbg_outs = {}

    def mkdbg(nm, shape):
        if nm not in dbg_outs:
            dbg_outs[nm] = nc.dram_tensor(f"dbg_{nm}", shape, f32,
                                          kind="ExternalOutput").ap()
        return dbg_outs[nm]

    with TileContext(nc, pool_alloc_mode="queue") as tc, ExitStack() as ctx:
        # Pin the activation table to natural_log_exp_and_others (set 6):
        # it contains exp/ln/square/relu/copy/identity, i.e. every ACT
        # function this program uses, so no further table loads are needed.
        nc.scalar.add_instruction(mybir.InstLoadActFuncSet(
            name=nc.get_next_instruction_name(), act_func_set_id=6,
            ins=[], outs=[]))
        pers = ctx.enter_context(tc.tile_pool(name="pers", bufs=1))
        # ---- persistent small constants -------------------------------
        stair = pers.tile([P, P], bf16, name="stair_t")
        nc.gpsimd.dma_start(out=stair, in_=staird)
        # selector for the head-pair 1/n broadcast: row 0 -> partitions
        # 0:64 (even head), row 32 -> partitions 64:128 (odd head)
        # head-pair 1/n machinery: denominators for pairs (0,1) land on
        # rows 0/32/64/96 of tile A, pairs (2,3) on tile B (32-aligned
        # partition writes only).  One DVE reciprocal+cast per tile.
        # selAB col block hp%2 maps rows (0,32) or (64,96) onto the
        # 64-partition halves of a pair's output.
        selAB = pers.tile([97, 2 * P], bf16, name="selAB_t")
        nc.vector.memset(selAB, 0.0)
        nc.gpsimd.memset(selAB[0:1, 0:DH], 1.0)
        nc.gpsimd.memset(selAB[32:33, DH:P], 1.0)
        nc.gpsimd.memset(selAB[64:65, P:P + DH], 1.0)
        nc.gpsimd.memset(selAB[96:97, P + DH:2 * P], 1.0)
        nden97 = [pers.tile([97, 512], f32, name=f"nden97_{i}")
                  for i in range(2)]
        for t in nden97:
            nc.vector.memset(t, 1.0)
        nrec97 = [pers.tile([97, 512], f32, name=f"nrec97_{i}")
                  for i in range(2)]
        ninv97 = [pers.tile([97, 512], bf16, name=f"ninv97_{i}")
                  for i in range(2)]
        ccbias = pers.tile([P, 1], f32, name="ccbias_t")
        nc.gpsimd.dma_start(out=ccbias, in_=ccbias_d)
        ctbias = pers.tile([P, 1], f32, name="ctbias_t")
        nc.gpsimd.dma_start(out=ctbias, in_=ctbias_d)
        kwbias = pers.tile([KW, 1], f32, name="kwbias_t")
        nc.gpsimd.dma_start(out=kwbias, in_=kwbias_d)
        gwa_t = pers.tile([P, NCH], bf16, name="gwa_t")
        nc.gpsimd.dma_start(out=gwa_t,
                            in_=gwA.rearrange("(i p) o -> p i o", p=P))
        gwb_t = pers.tile([P, NCH], bf16, name="gwb_t")
        nc.gpsimd.dma_start(out=gwb_t,
                            in_=gwB.rearrange("(i p) o -> p i o", p=P))
        ones_b = pers.tile([P, 1], bf16, name="ones_b")
        nc.vector.memset(ones_b, 1.0)
        ones_row = pers.tile([1, P], bf16, name="ones_row")
        nc.vector.memset(ones_row, 1.0)
        eps_t = pers.tile([1, 1], f32, name="eps_t")
        nc.vector.memset(eps_t, 1e-5)
        gb_t = pers.tile([1, 1], f32, name="gb_t")
        nc.vector.memset(gb_t, -float(gate_b))
        affine = None
        if apply_affine:
            affine = pers.tile([P, NCH * 8], f32, name="affine_t")
            nc.sync.dma_start(out=affine, in_=affine_d)

        def tap(nm, tiles):
            if not debug:
                return
            cols = tiles[0].shape[-1]
            d = mkdbg(nm, [len(tiles) * P, cols])
            for i, t in enumerate(tiles):
                rows = t.shape[0]
                nc.gpsimd.dma_start(out=d[i * P:i * P + rows, :], in_=t)

        # ---- global shared pools --------------------------------------
        # residual/LN-out tiles, reused across stages via shared tags
        rpool = ctx.enter_context(tc.tile_pool(name="rpool", bufs=1))

        def mktiles(nm, cols=512, dt=f32, n=NCH, tagp=None):
            tagp = tagp or nm
            return [rpool.tile([P, cols], dt, name=f"{nm}{i}", tag=f"{tagp}{i}",
                               bufs=1) for i in range(n)]

        # small 1/8-partition tiles + broadcast tiles, shared by all stages
        smallp = ctx.enter_context(tc.tile_pool(name="smallp", bufs=1))
        # transient [128, *] tiles (exp outputs, LN scratch, gate scratch)
        trp = ctx.enter_context(tc.tile_pool(name="trp", bufs=1))
        # PSUM: pps = projection/V accumulators; x_ps = paired scores
        # (2 banks each); x_po = AV out + LN stats + gate
        psA = ctx.enter_context(tc.tile_pool(name="psA", bufs=2, space="PSUM"))
        psB = ctx.enter_context(tc.tile_pool(name="psB", bufs=2, space="PSUM"))

        def load_w(pool, names):
            for n in names:
                if n in W8NAMES:
                    fp8 = mybir.dt.float8e4
                    wt[n] = pool.tile([P, 4 * D], fp8, name=f"{n}_t",
                                      tag=f"{n}_t", bufs=1)
                    nc.sync.dma_start(
                        out=wt[n].rearrange("p (c o n) -> p c o n",
                                            c=2, o=2),
                        in_=wd[n].rearrange("(c o p) n -> p c o n",
                                            c=2, o=2, p=P))
                else:
                    wt[n] = pool.tile([P, NCH * D], bf16, name=f"{n}_t",
                                      tag=f"{n}_t", bufs=1)
                    nc.sync.dma_start(
                        out=wt[n].rearrange("p (i n) -> p i n", n=D),
                        in_=wd[n].rearrange("(i p) n -> p i n", p=P))
        wt = {}

        def w8_lhsT(n, c, j):
            return wt[n].rearrange("p (c o n) -> p c o n",
                                   c=2, o=2)[:, c, :, j * P:(j + 1) * P]

        def w8_rhs(n, c):
            return wt[n].rearrange("p (c o n) -> p c o n", c=2, o=2)[:, c]

        def w8_flat(n, c, o, j):
            return wt[n].rearrange("p (c o n) -> p c o n",
                                   c=2, o=2)[:, c, o, j * P:(j + 1) * P]

        def load_act8(pool, nm, dram_ap, cols):
            fp8 = mybir.dt.float8e4
            tiles = []
            for c in range(2):
                t = pool.tile([P, 2 * cols], fp8, name=f"{nm}{c}",
                              tag=f"{nm}{c}", bufs=1)
                nc.sync.dma_start(
                    out=t.rearrange("p (o n) -> p o n", o=2),
                    in_=dram_ap[c * 2 * P:(c + 1) * 2 * P, :].rearrange(
                        "(o p) n -> p o n", o=2, p=P))
                tiles.append(t.rearrange("p (o n) -> p o n", o=2))
            return tiles

        def w_lhsT(n, i, j):
            return wt[n][:, i * D + j * P: i * D + (j + 1) * P]

        def w_rhs(n, i, cols=D):
            return wt[n][:, i * D: i * D + cols]

        def load_act(pool, nm, dram_ap, cols):
            tiles = []
            for i in range(NCH):
                t = pool.tile([P, cols], bf16, name=f"{nm}{i}",
                              tag=f"{nm}{i}", bufs=1)
                nc.sync.dma_start(out=t, in_=dram_ap[i * P:(i + 1) * P, :])
                tiles.append(t)
            return tiles

        # ----------------------------------------------------------------
        # helpers
        # ----------------------------------------------------------------
        def proj_fm_groups(wn, rhs_tiles, ncols, out_tiles, evict):
            """Per-psum-group closures for a feature-major projection; each
            emits 4 accumulating matmuls + one eviction."""
            ntt = (ncols + 511) // 512
            groups = []
            for j in range(NCH):
                for t in range(ntt):
                    def g(j=j, t=t):
                        cs = t * 512
                        ce = min(ncols, cs + 512)
                        ps = psA.tile([P, ce - cs], mybir.dt.float32,
                                      name="proj_ps", tag="pps")
                        for i in range(NCH):
                            nc.tensor.matmul(ps, w_lhsT(wn, i, j),
                                             rhs_tiles[i][:, cs:ce],
                                             start=(i == 0),
                                             stop=(i == NCH - 1))
                        evict(j, cs, ce, ps, out_tiles)
                    groups.append(g)
            return groups

        def proj_fm(wn, rhs_tiles, ncols, out_tiles, evict):
            for g in proj_fm_groups(wn, rhs_tiles, ncols, out_tiles, evict):
                g()

        DR = mybir.MatmulPerfMode.DoubleRow
        DESC = 1.0 / 1024.0  # descale: activation x16, weight x64

        def evict_ds(j, cs, ce, ps, out_tiles, balance=False):
            if balance and (j + cs // 512) % 2 == 1:
                nc.scalar.mul(out_tiles[j][:, cs:ce], ps, DESC)
            else:
                nc.vector.tensor_scalar_mul(out_tiles[j][:, cs:ce], ps, DESC)

        def proj_fm8_groups(wn, rhs8, ncols, out_tiles, balance=False):
            """fp8 DoubleRow feature-major projection (descaled evict).
            Falls back to normal-mode fp8 matmuls when ncols < 128."""
            ntt = (ncols + 511) // 512
            groups = []
            for j in range(NCH):
                for t in range(ntt):
                    def g(j=j, t=t):
                        cs = t * 512
                        ce = min(ncols, cs + 512)
                        ps = psA.tile([P, ce - cs], mybir.dt.float32,
                                      name="proj_ps", tag="pps")
                        if ncols >= P:
                            for c in range(2):
                                nc.tensor.matmul(
                                    ps, w8_lhsT(wn, c, j),
                                    rhs8[c][:, :, cs:ce],
                                    start=(c == 0), stop=(c == 1),
                                    perf_mode=DR)
                        else:
                            for ci in range(4):
                                c, o = ci // 2, ci % 2
                                nc.tensor.matmul(
                                    ps, w8_flat(wn, c, o, j),
                                    rhs8[c][:, o, cs:ce],
                                    start=(ci == 0), stop=(ci == 3))
                        evict_ds(j, cs, ce, ps, out_tiles, balance)
                    groups.append(g)
            return groups

        def proj_v8_groups(enc8, wn, nkv, vt_list, vpool, ktag):
            nch_tok = (nkv + P - 1) // P
            vt_list.extend(
                vpool.tile([min(P, nkv - m * P), H * (DH + 1)], bf16,
                           name=f"{ktag}_v{m}", tag=f"{ktag}_v{m}", bufs=1)
                for m in range(nch_tok))
            groups = []
            for m in range(nch_tok):
                def g(m=m):
                    rows = vt_list[m].shape[0]
                    ps = psA.tile([rows, D], mybir.dt.float32,
                                  name="v_ps", tag="pps")
                    for c in range(2):
                        nc.tensor.matmul(ps,
                                         enc8[c][:, :, m * P:m * P + rows],
                                         w8_rhs(wn, c),
                                         start=(c == 0), stop=(c == 1),
                                         perf_mode=DR)
                    vt = vt_list[m]
                    src3 = ps.rearrange("p (g c) -> p g c", c=DH)
                    dst3 = vt.rearrange("p (g c) -> p g c", c=DH + 1)
                    nc.vector.tensor_scalar_mul(dst3[:, :, 0:DH], src3, DESC)
                    nc.gpsimd.memset(dst3[:, :, DH:DH + 1], 1.0)
                groups.append(g)
            return groups

        def evict_copy(j, cs, ce, ps, out_tiles):
            nc.vector.tensor_copy(out_tiles[j][:, cs:ce], ps)

        def evict_copy_bal(j, cs, ce, ps, out_tiles):
            if (j + cs // 512) % 2 == 0:
                nc.vector.tensor_copy(out_tiles[j][:, cs:ce], ps)
            else:
                nc.scalar.copy(out_tiles[j][:, cs:ce], ps)

        def proj_v_groups(enc_tiles, wn, nkv, vt_list, vpool, ktag):
            nch_tok = (nkv + P - 1) // P
            vt_list.extend(
                vpool.tile([min(P, nkv - m * P), H * (DH + 1)], bf16,
                           name=f"{ktag}_v{m}", tag=f"{ktag}_v{m}", bufs=1)
                for m in range(nch_tok))
            groups = []
            for m in range(nch_tok):
                def g(m=m):
                    rows = vt_list[m].shape[0]
                    ps = psA.tile([rows, D], mybir.dt.float32,
                                  name="v_ps", tag="pps")
                    for i in range(NCH):
                        nc.tensor.matmul(ps,
                                         enc_tiles[i][:, m * P:m * P + rows],
                                         w_rhs(wn, i),
                                         start=(i == 0), stop=(i == NCH - 1))
                    vt = vt_list[m]
                    src3 = ps.rearrange("p (g c) -> p g c", c=DH)
                    dst3 = vt.rearrange("p (g c) -> p g c", c=DH + 1)
                    nc.vector.tensor_copy(dst3[:, :, 0:DH], src3)
                    nc.gpsimd.memset(dst3[:, :, DH:DH + 1], 1.0)
                groups.append(g)
            return groups

        def proj_v(enc_tiles, wn, nkv, vt_list, vpool, ktag):
            for g in proj_v_groups(enc_tiles, wn, nkv, vt_list, vpool, ktag):
                g()

        def attention(qt, kt, vt_list, out_tiles, bias_tile, causal, ktag,
                      fillers=None):
            """Multi-head attention.  Head pairs share one [rows,1024]
            scores psum + one merged exp; the AV matmul for tile kt is
            emitted after the scores matmul for tile kt+1 so the ACT exp
            overlaps PE work.  Causal scores/AV are restricted to the
            unmasked column range.  The softmax normalization runs per
            head-pair (collect denominators on partitions 0/32, one DVE
            reciprocal, one K=33 selector broadcast, one fused multiply)
            so it hides under the next pair's kt loop.  `fillers` is a
            list of closures emitting independent PE work; one is popped
            after each kt iteration to fill the exp-wait bubbles."""
            nkt = len(vt_list)
            fillers = fillers if fillers is not None else []

            def pop_filler():
                if fillers:
                    fillers.pop(0)()

            for hp in range(H // 2):
                po = []
                for s in range(2):
                    po.append(psB.tile([DH + 1, 512], mybir.dt.float32,
                                       name=f"{ktag}_po{s}", tag="x_po"))
                pend = None  # deferred AV: (kt_i, pt2, c0)

                def flush_av(last):
                    kt_i, pt2, c0 = pend
                    for s in range(2):
                        h = 2 * hp + s
                        nc.tensor.matmul(
                            po[s][:, c0:512],
                            vt_list[kt_i][:, h * (DH + 1):
                                          (h + 1) * (DH + 1)],
                            pt2[:, s * 512 + c0:(s + 1) * 512],
                            start=(kt_i == 0), stop=last)

                for kt_i in range(nkt):
                    rows = vt_list[kt_i].shape[0]
                    d = kt_i - (nkt - 4) if causal else -1
                    c0 = d * P if (causal and d > 0) else 0
                    ps2 = psB.tile([rows, 1024], mybir.dt.float32,
                                   name=f"{ktag}_ps", tag="x_ps")
                    pt2 = trp.tile([rows, 1024], bf16,
                                   name=f"{ktag}_pt", tag="pt", bufs=3)
                    for s in range(2):
                        ro = s * DH
                        o = s * 512
                        nc.tensor.matmul(
                            ps2[:, o + c0:o + 512],
                            kt[hp][ro:ro + DH, kt_i * P:kt_i * P + rows],
                            qt[hp][ro:ro + DH, c0:512], start=True, stop=True)
                    if causal and d >= 0:
                        for s in range(2):
                            o = s * 512
                            nc.scalar.activation(pt2[:, o + c0:o + 512],
                                                 ps2[:, o + c0:o + 512],
                                                 AF.Exp, scale=0.125)
                            nc.vector.tensor_mul(
                                pt2[:, o + d * P:o + (d + 1) * P],
                                pt2[:, o + d * P:o + (d + 1) * P], stair)
                    else:
                        bias = 0.0
                        if bias_tile is not None and kt_i == nkt - 1:
                            bias = bias_tile[:rows, :]
                        nc.scalar.activation(pt2, ps2, AF.Exp,
                                             bias=bias, scale=0.125)
                    if pend is not None:
                        flush_av(False)
                        if kt_i % 2 == 1:
                            pop_filler()
                    pend = (kt_i, pt2, c0)
                flush_av(True)
                # stage this pair's denominators into tile A (pairs 0,1)
                # or B (pairs 2,3) on 32-aligned partitions
                ab, r0 = hp // 2, (hp % 2) * DH
                nc.vector.tensor_copy(nden97[ab][r0:r0 + 1, :],
                                      po[0][DH:DH + 1, :])
                nc.vector.tensor_copy(nden97[ab][r0 + 32:r0 + 33, :],
                                      po[1][DH:DH + 1, :])
                nc.scalar.copy(out_tiles[hp][0:DH, :], po[0][0:DH, :])
                nc.vector.tensor_copy(out_tiles[hp][DH:P, :], po[1][0:DH, :])
                if hp % 2 == 1:
                    # both pairs of this tile staged: one reciprocal+cast,
                    # then normalize both pairs (overlaps the next loop)
                    nc.vector.reciprocal_approx_fast(out=nrec97[ab],
                                                     in_=nden97[ab])
                    nc.vector.tensor_copy(ninv97[ab], nrec97[ab])
                    for hq in (hp - 1, hp):
                        pop_filler()
                        nb = psA.tile([P, 512], mybir.dt.float32,
                                      name=f"{ktag}_nb{hq}", tag="pps")
                        nc.tensor.matmul(nb,
                                         selAB[:, (hq % 2) * P:
                                               (hq % 2 + 1) * P],
                                         ninv97[ab], start=True, stop=True)
                        nc.vector.tensor_mul(out_tiles[hq], out_tiles[hq],
                                             nb)
            for g in fillers:
                g()

        def layernorm(r_tiles, out_tiles, ln_idx, mid=None):
            sq = [trp.tile([P, 512], bf16, name=f"ln{ln_idx}_sq", tag="ln_sq",
                           bufs=2) for _ in range(NCH)]
            for j in range(NCH):
                nc.scalar.activation(sq[j], r_tiles[j], AF.Square)
            ps_s = psB.tile([1, 512], mybir.dt.float32,
                            name="ln_ps_s", tag="x_po")
            ps_q = psB.tile([1, 512], mybir.dt.float32,
                            name="ln_ps_q", tag="x_po")
            for j in range(NCH):
                nc.tensor.matmul(ps_s, ones_b, r_tiles[j],
                                 start=(j == 0), stop=(j == NCH - 1))
            for j in range(NCH):
                nc.tensor.matmul(ps_q, ones_b, sq[j],
                                 start=(j == 0), stop=(j == NCH - 1))
            if mid is not None:
                mid()
            mean16 = smallp.tile([1, 512], bf16,
                                 name="ln_mean16", tag="ln_stat", bufs=3)
            nc.vector.tensor_scalar_mul(mean16, ps_s, 1.0 / D)
            meanb = psB.tile([P, 512], mybir.dt.float32,
                             name="ln_meanb", tag="x_po")
            nc.tensor.matmul(meanb, ones_row, mean16, start=True, stop=True)
            msq = smallp.tile([1, 512], mybir.dt.float32,
                              name="ln_msq", tag="ln_stat", bufs=3)
            nc.scalar.activation(msq, ps_s, AF.Square, scale=1.0 / D)
            var = smallp.tile([1, 512], mybir.dt.float32,
                              name="ln_var", tag="ln_stat", bufs=3)
            nc.vector.scalar_tensor_tensor(var, ps_q, 1.0 / D, msq,
                                           op0=OP.mult, op1=OP.subtract)
            lnv = smallp.tile([1, 512], mybir.dt.float32,
                              name="ln_lnv", tag="ln_stat", bufs=3)
            nc.scalar.activation(lnv, var, AF.Ln, bias=eps_t[:, :])
            rstd = smallp.tile([1, 512], bf16,
                               name="ln_rstd", tag="ln_stat", bufs=3)
            nc.scalar.activation(rstd, lnv, AF.Exp, scale=-0.5)
            rstdb = psB.tile([P, 512], mybir.dt.float32,
                             name="ln_rstdb", tag="x_po")
            nc.tensor.matmul(rstdb, ones_row, rstd, start=True, stop=True)
            for j in range(NCH):
                tmp = trp.tile([P, 512], bf16,
                               name="ln_tmp", tag="ln_tmp", bufs=2)
                nc.vector.tensor_sub(tmp, r_tiles[j], meanb)
                nc.vector.tensor_mul(out_tiles[j], tmp, rstdb)
                if apply_affine:
                    g = affine[:, ln_idx * 2 * NCH + j: ln_idx * 2 * NCH + j + 1]
                    b = affine[:, ln_idx * 2 * NCH + NCH + j:
                               ln_idx * 2 * NCH + NCH + j + 1]
                    nc.vector.tensor_scalar(out_tiles[j], out_tiles[j],
                                            g, b, op0=OP.mult, op1=OP.add)

        # ================================================================
        # emission (ordered for cross-stage overlap)
        # ================================================================
        r1 = mktiles("r1", dt=bf16, tagp="rA")
        y = mktiles("y", dt=bf16, tagp="lnA")
        r2 = mktiles("r2", dt=bf16, tagp="rB")
        z = mktiles("z", dt=bf16, tagp="lnB")
        r3 = None  # allocated after r1 dies
        ze = None

        # ct pool created first so it outlives ccsb (LIFO pool stack);
        # its DMA loads are issued after LN1 and overlap the cc/ck stage
        ctsb = ctx.enter_context(tc.tile_pool(name="tail_sb", bufs=1))
        ccsb_cm = tc.tile_pool(name="cc_sb", bufs=1)
        ccsb = ccsb_cm.__enter__()
        sasb_cm = tc.tile_pool(name="sa_sb", bufs=1)
        sasb = sasb_cm.__enter__()

        # --- stage 1: self attention (fp8 DoubleRow K/V/Q) ---
        load_w(sasb, ["sa_wk"])
        xkv8 = load_act8(sasb, "xkv8", xkv8T, KV)
        load_w(sasb, ["sa_wv", "sa_wq"])
        xq = load_act(sasb, "xq", xqT, 512)
        load_w(sasb, ["sa_wo"])
        qt = [sasb.tile([P, 512], bf16, name=f"sa_q{i}", tag=f"sa_q{i}",
                        bufs=1) for i in range(NCH)]
        ktl = [sasb.tile([P, KV], bf16, name=f"sa_k{i}", tag=f"sa_k{i}",
                         bufs=1) for i in range(NCH)]
        xq8 = [t[:, :, QOFF:QOFF + 512] for t in xkv8]
        for g in proj_fm8_groups("sa_wk", xkv8, KV, ktl, balance=True):
            g()
        vts = []
        for g in proj_v8_groups(xkv8, "sa_wv", KV, vts, sasb, "sa"):
            g()
        for g in proj_fm8_groups("sa_wq", xq8, 512, qt, balance=True):
            g()
        at = [trp.tile([P, 512], bf16, name=f"sa_at{i}", tag=f"at{i}",
                       bufs=1) for i in range(NCH)]
        # cc K/V projections are independent of sa: interleave them into
        # sa's kt loops as PE fillers (their DMA loads were issued above)
        load_w(ccsb, ["cc_wk", "cc_wv", "ck_wk", "ck_wv",
                      "cc_wq", "ck_wq", "cc_wo", "ck_wo"])
        srcl8 = load_act8(ccsb, "src8", src8T, kts_cc * P)
        kwe8 = load_act8(ccsb, "kw8", kw8T, KW)
        cc_kt = [ccsb.tile([P, kts_cc * P], bf16, name=f"cc_k{i}",
                           tag=f"cc_k{i}", bufs=1) for i in range(NCH)]
        cc_vts = []
        sa_fill = (proj_fm8_groups("cc_wk", srcl8, kts_cc * P, cc_kt)
                   + proj_v8_groups(srcl8, "cc_wv", kts_cc * P, cc_vts,
                                    ccsb, "cc"))
        attention(qt, ktl, vts, at, None, True, "sa", fillers=sa_fill)

        def evict_resid_x(j, cs, ce, ps, out_tiles):
            nc.vector.tensor_add(out_tiles[j][:, cs:ce], ps, xq[j])
        tap("sa_at", at)
        proj_fm("sa_wo", at, 512, r1, evict_resid_x)
        tap("r1", r1)
        ck_kt = [ccsb.tile([P, KW], bf16, name=f"ck_k{i}", tag=f"ck_k{i}",
                           bufs=1) for i in range(NCH)]
        ck_vts = []

        def ln1_mid():
            for g in proj_fm8_groups("ck_wk", kwe8, KW, ck_kt):
                g()
            for g in proj_v8_groups(kwe8, "ck_wv", KW, ck_vts, ccsb, "ck"):
                g()
        layernorm(r1, y, 0, mid=ln1_mid)
        tap("y", y)
        sasb_cm.__exit__(None, None, None)

        # ct weight/activation DMA loads overlap the whole cc/ck stage
        load_w(ctsb, ["ct_wk", "ct_wv", "ct_wq", "ct_wo"])
        tmpl8 = load_act8(ctsb, "tmpl8", tmpl8T, kts_ct * P)

        # --- stage 2: cc + ck cross attention + gate ---
        cc_qt = [ccsb.tile([P, 512], bf16, name=f"cc_q{i}", tag=f"cc_q{i}",
                           bufs=1) for i in range(NCH)]
        proj_fm("cc_wq", y, 512, cc_qt, evict_copy_bal)
        cc_at = [trp.tile([P, 512], bf16, name=f"cc_at{i}", tag=f"at{i}",
                          bufs=1) for i in range(NCH)]
        ck_qt = [ccsb.tile([P, 512], bf16, name=f"ck_q{i}", tag=f"ck_q{i}",
                           bufs=1) for i in range(NCH)]
        cc_fill = proj_fm_groups("ck_wq", y, 512, ck_qt, evict_copy)
        attention(cc_qt, cc_kt, cc_vts, cc_at, ccbias, False, "cc",
                  fillers=cc_fill)
        ck_at = [trp.tile([P, 512], bf16, name=f"ck_at{i}", tag=f"ckat{i}",
                          bufs=1) for i in range(NCH)]
        y2c = [ccsb.tile([P, 512], bf16, name=f"y2c{i}", tag=f"y2c{i}",
                         bufs=1) for i in range(NCH)]
        ck_fill = proj_fm_groups("cc_wo", cc_at, 512, y2c, evict_copy)
        attention(ck_qt, ck_kt, ck_vts, ck_at, kwbias, False, "ck",
                  fillers=ck_fill)
        y2k = [ccsb.tile([P, 512], bf16, name=f"y2k{i}", tag=f"y2k{i}",
                         bufs=1) for i in range(NCH)]
        proj_fm("ck_wo", ck_at, 512, y2k, evict_copy_bal)

        # --- gate ---
        ps_g = psB.tile([1, 512], mybir.dt.float32, name="gate_ps",
                        tag="x_po")
        for i in range(NCH):
            nc.tensor.matmul(ps_g, gwa_t[:, i:i + 1], y2c[i],
                             start=(i == 0), stop=False)
        for i in range(NCH):
            nc.tensor.matmul(ps_g, gwb_t[:, i:i + 1], y2k[i],
                             start=False, stop=(i == NCH - 1))
        # g-independent combine pieces, overlap the gate ACT/DVE chain
        gdt = [trp.tile([P, 512], bf16, name=f"gate_dt{j}", tag=f"gate_dt{j}",
                        bufs=1) for j in range(NCH)]
        for j in range(NCH):
            nc.vector.tensor_sub(gdt[j], y2c[j], y2k[j])
            nc.vector.tensor_add(r2[j], y[j], y2k[j])
        ct_kt = [ctsb.tile([P, kts_ct * P], bf16, name=f"ct_k{i}",
                           tag=f"ct_k{i}", bufs=1) for i in range(NCH)]

        def ct_mid():
            for g in proj_fm8_groups("ct_wk", tmpl8, kts_ct * P, ct_kt):
                g()
        ge = smallp.tile([1, 512], mybir.dt.float32, name="gate_e",
                         tag="gate_edg", bufs=2)
        nc.scalar.activation(ge, ps_g, AF.Exp, scale=-1.0, bias=gb_t[:, :])
        gp1 = smallp.tile([1, 512], mybir.dt.float32, name="gate_p1",
                          tag="gate_edg", bufs=2)
        nc.vector.tensor_scalar_add(gp1, ge, 1.0)
        grc = smallp.tile([1, 512], mybir.dt.float32, name="gate_rc",
                          tag="gate_edg", bufs=2)
        nc.vector.reciprocal_approx_fast(out=grc, in_=gp1)
        gg = smallp.tile([1, 512], bf16, name="gate_g",
                         tag="gate_edg", bufs=2)
        nc.vector.tensor_copy(gg, grc)
        ct_mid()
        ggb = psB.tile([P, 512], mybir.dt.float32, name="gate_gb",
                       tag="x_po")
        nc.tensor.matmul(ggb, ones_row, gg, start=True, stop=True)
        # r2 = (y + y2k) + g*(y2c - y2k)
        for j in range(NCH):
            nc.vector.tensor_mul(gdt[j], gdt[j], ggb)
            nc.vector.tensor_add(r2[j], r2[j], gdt[j])
        tap("y2c", y2c)
        tap("y2k", y2k)
        tap("r2", r2)
        ccsb_cm.__exit__(None, None, None)
        # FFN weights: DMA overlaps the ct attention stage
        ffsb = ctx.enter_context(tc.tile_pool(name="ff_sb", bufs=1))
        w1t = ffsb.tile([P, NCH * DFF], bf16, name="w1_t", tag="w1_t")
        nc.sync.dma_start(out=w1t.rearrange("p (i n) -> p i n", n=DFF),
                          in_=w1d.rearrange("(i p) n -> p i n", p=P))
        w2t = ffsb.tile([P, (DFF // P) * D], bf16, name="w2_t", tag="w2_t")
        nc.sync.dma_start(out=w2t.rearrange("p (i n) -> p i n", n=D),
                          in_=w2d.rearrange("(i p) n -> p i n", p=P))
        ct_vts = []

        def ln2_mid():
            for g in proj_v8_groups(tmpl8, "ct_wv", kts_ct * P, ct_vts,
                                    ctsb, "ct"):
                g()
        layernorm(r2, z, 1, mid=ln2_mid)
        tap("z", z)

        # --- stage 3: ct cross attention ---
        r3 = mktiles("r3", dt=bf16, tagp="rA")
        ze = mktiles("ze", dt=bf16, tagp="lnA")
        ct_qt = [ffsb.tile([P, 512], bf16, name=f"ct_q{i}", tag=f"ct_q{i}",
                           bufs=1) for i in range(NCH)]
        proj_fm("ct_wq", z, 512, ct_qt, evict_copy_bal)
        ct_at = [trp.tile([P, 512], bf16, name=f"ct_at{i}", tag=f"at{i}",
                          bufs=1) for i in range(NCH)]
        attention(ct_qt, ct_kt, ct_vts, ct_at, ctbias, False, "ct")

        def evict_resid_r2(j, cs, ce, ps, out_tiles):
            nc.vector.tensor_add(out_tiles[j][:, cs:ce], ps, z[j])
        tap("ct_at", ct_at)
        proj_fm("ct_wo", ct_at, 512, r3, evict_resid_r2)
        tap("r3", r3)
        layernorm(r3, ze, 2)
        tap("ze", ze)

        # --- stage 4: FFN ---
        ht = [ffsb.tile([P, 512], bf16, name=f"ff_h{i}", tag=f"ff_h{i}",
                        bufs=1) for i in range(DFF // P)]
        for jf in range(DFF // P):
            ps = psA.tile([P, 512], mybir.dt.float32, name="ff_ps",
                          tag="pps")
            for i in range(NCH):
                nc.tensor.matmul(ps, w1t[:, i * DFF + jf * P:
                                         i * DFF + (jf + 1) * P],
                                 ze[i], start=(i == 0), stop=(i == NCH - 1))
            if jf % 2 == 0:
                nc.scalar.activation(ht[jf], ps, AF.Relu)
            else:
                nc.vector.tensor_scalar_max(ht[jf], ps, 0.0)
        r4 = mktiles("r4", dt=bf16, tagp="rB")
        for j in range(NCH):
            ps = psA.tile([P, 512], mybir.dt.float32, name="ff_ps2",
                          tag="pps")
            for i in range(DFF // P):
                nc.tensor.matmul(ps, w2t[:, i * D + j * P: i * D + (j + 1) * P],
                                 ht[i], start=(i == 0),
                                 stop=(i == DFF // P - 1))
            nc.vector.tensor_add(r4[j], ps, ze[j])
        fin = [trp.tile([P, 512], bf16, name=f"fin{i}",
                        tag=f"at{i}", bufs=1) for i in range(NCH)]
        layernorm(r4, fin, 3)
        for j in range(NCH):
            nc.sync.dma_start(out=outT[j * P:(j + 1) * P, :], in_=fin[j])

    nc.compile()
    return nc


# ---------------------------------------------------------------------------
# host-side input preparation
# ---------------------------------------------------------------------------

W8NAMES_H = ("sa_wk", "sa_wv", "sa_wq", "cc_wk", "cc_wv",
             "ck_wk", "ck_wv", "ct_wk", "ct_wv")
FP8 = ml_dtypes.float8_e4m3
SW8, SX8 = 64.0, 16.0


def _prep_shared(inputs):
    """Cast/transform weights shared by every core."""
    sh = {}
    for n in ("sa", "cc", "ct", "ck"):
        for p in ("wq", "wk", "wv", "wo"):
            nm = f"{n}_{p}"
            if nm in W8NAMES_H:
                sh[nm] = np.ascontiguousarray(
                    np.clip(inputs[nm].astype(F32) * SW8,
                            -240, 240).astype(FP8))
            else:
                sh[nm] = np.ascontiguousarray(inputs[nm].astype(BF16))
    sh["ffn_w1"] = np.ascontiguousarray(inputs["ffn_w1"].astype(BF16))
    sh["ffn_w2"] = np.ascontiguousarray(inputs["ffn_w2"].astype(BF16))
    gw = inputs["gate_w"].astype(F32)
    sh["gwA"] = np.ascontiguousarray(gw[:D].astype(BF16))
    sh["gwB"] = np.ascontiguousarray(gw[D:].astype(BF16))
    kl, ql = np.arange(P)[:, None], np.arange(P)[None, :]
    sh["stair"] = np.where(kl <= ql, 1.0, 0.0).astype(BF16)
    return sh


def _len_bias(L, kts, width=P):
    """[width,1] f32 additive bias for the LAST kv tile."""
    base = (kts - 1) * P
    idx = base + np.arange(width)
    return np.where(idx < L, 0.0, NEG).astype(F32)[:, None]


def _q8(a):
    return np.clip(a.astype(F32) * SX8, -240, 240).astype(FP8)


def _prep_core(inputs, sh, b, qh, kts_cc, kts_ct):
    KVn = 512 * (qh + 1)
    m = dict(sh)
    xT = inputs["x"][b].T.astype(F32)  # [D, T]
    m["xkv8T"] = np.ascontiguousarray(_q8(xT[:, :KVn]))
    m["xqT"] = np.ascontiguousarray(
        xT[:, qh * 512:(qh + 1) * 512].astype(BF16))
    Ls = int(inputs["source_code_len"][b])
    st = np.zeros((D, kts_cc * P), FP8)
    st[:, :Ls] = _q8(inputs["source_code_enc"][b, :Ls].T)
    m["src8T"] = st
    Lt = int(inputs["template_len"][b])
    tt = np.zeros((D, kts_ct * P), FP8)
    tt[:, :Lt] = _q8(inputs["template_enc"][b, :Lt].T)
    m["tmpl8T"] = tt
    m["kw8T"] = np.ascontiguousarray(_q8(inputs["keywords_enc"][b].T))
    m["cc_bias"] = _len_bias(Ls, kts_cc)
    m["ct_bias"] = _len_bias(Lt, kts_ct)
    m["kw_bias"] = _len_bias(int(inputs["keywords_len"][b]), 1, KW)
    return m


# ---------------------------------------------------------------------------
# concurrent multi-program PJRT runner (adapted from bass2jax.run_bass_via_pjrt)
# ---------------------------------------------------------------------------

def _run_groups(groups):
    """groups: list of (nc, core_ids, in_maps).  Dispatch all groups onto
    their own device subsets, then gather.  Returns {core_id: {name: arr}}."""
    import jax
    import numpy as _np
    from jax.sharding import Mesh, PartitionSpec
    from jax.experimental.shard_map import shard_map
    from concourse import bass2jax
    from concourse.bass2jax import (_bass_exec_p, install_neuronx_cc_hook,
                                    partition_id_tensor)

    install_neuronx_cc_hook()
    devices = jax.devices()

    def make_launch(nc, core_ids, in_maps):
        pname = (nc.partition_id_tensor.name
                 if nc.partition_id_tensor else None)
        in_names, out_names, out_avals, zero_outs = [], [], [], []
        for alloc in nc.m.functions[0].allocations:
            if not isinstance(alloc, mybir.MemoryLocationSet):
                continue
            name = alloc.memorylocations[0].name
            if alloc.kind == "ExternalInput":
                if name == pname:
                    continue
                in_names.append(name)
            elif alloc.kind == "ExternalOutput":
                shape = tuple(alloc.tensor_shape)
                dtype = mybir.dt.np(alloc.dtype)
                out_names.append(name)
                out_avals.append(jax.core.ShapedArray(shape, dtype))
                zero_outs.append(_np.zeros(shape, dtype))
        n_params, n_outs = len(in_names), len(out_avals)
        all_in_names = in_names + out_names
        if pname is not None:
            all_in_names = all_in_names + [pname]

        def _body(*args):
            operands = list(args)
            if pname is not None:
                operands.append(partition_id_tensor())
            outs = _bass_exec_p.bind(
                *operands, out_avals=tuple(out_avals),
                in_names=tuple(all_in_names), out_names=tuple(out_names),
                lowering_input_output_aliases=(),
                sim_require_finite=False, sim_require_nnan=False, nc=nc)
            return tuple(outs)

        donate = tuple(range(n_params, n_params + n_outs))
        devs = [devices[c] for c in core_ids]
        if len(core_ids) == 1:
            fn = jax.jit(_body, donate_argnums=donate, keep_unused=True,
                         device=devs[0])
            args = [in_maps[0][nm] for nm in in_names] + list(zero_outs)
            out_arrs = fn(*args)
            return out_names, out_avals, out_arrs, None
        mesh = Mesh(_np.asarray(devs), ("core",))
        in_specs = (PartitionSpec("core"),) * (n_params + n_outs)
        out_specs = (PartitionSpec("core"),) * n_outs
        fn = jax.jit(shard_map(_body, mesh=mesh, in_specs=in_specs,
                               out_specs=out_specs, check_rep=False),
                     donate_argnums=donate, keep_unused=True)
        cat = [_np.concatenate([_np.asarray(m[nm]) for m in in_maps], axis=0)
               for nm in in_names]
        catz = [_np.zeros((len(core_ids) * z.shape[0], *z.shape[1:]), z.dtype)
                for z in zero_outs]
        out_arrs = fn(*cat, *catz)
        return out_names, out_avals, out_arrs, len(core_ids)

    last_err = None
    for _attempt in range(3):
        try:
            launched = []
            for nc, core_ids, in_maps in groups:
                launched.append((core_ids, make_launch(nc, core_ids, in_maps)))
            results = {}
            for core_ids, (out_names, out_avals, out_arrs, ncores) in launched:
                if ncores is None:
                    results[core_ids[0]] = {nm: _np.asarray(out_arrs[i])
                                            for i, nm in enumerate(out_names)}
                else:
                    for ci, c in enumerate(core_ids):
                        results[c] = {
                            nm: _np.asarray(out_arrs[i]).reshape(
                                ncores, *out_avals[i].shape)[ci]
                            for i, nm in enumerate(out_names)}
            return results
        except Exception as e:  # transient NRT device errors: retry
            last_err = e
            import time as _time
            _time.sleep(2.0)
    raise last_err


_PROGRAM_CACHE = {}
_CACHE_LOCK = threading.Lock()


def _get_program(key):
    with _CACHE_LOCK:
        if key in _PROGRAM_CACHE:
            return _PROGRAM_CACHE[key]
    qh, kts_cc, kts_ct, gate_b, aff = key
    nc = build_program(qh, kts_cc, kts_ct, gate_b=gate_b, apply_affine=aff)
    with _CACHE_LOCK:
        _PROGRAM_CACHE[key] = nc
    return nc


# ---------------------------------------------------------------------------
# entry point
# ---------------------------------------------------------------------------

def kernel(**inputs):
    inputs = {k: np.asarray(v) for k, v in inputs.items()}
    gate_b = float(inputs["gate_b"].reshape(-1)[0])
    aff = not all(
        np.all(inputs[f"ln{j}_g"] == 1.0) and np.all(inputs[f"ln{j}_b"] == 0.0)
        for j in range(1, 5))
    affine_arr = None
    if aff:
        affine_arr = np.zeros((P, NCH * 8), F32)
        for ln in range(4):
            g = inputs[f"ln{ln + 1}_g"].astype(F32).reshape(NCH, P).T
            bb = inputs[f"ln{ln + 1}_b"].astype(F32).reshape(NCH, P).T
            affine_arr[:, ln * 2 * NCH: ln * 2 * NCH + NCH] = g
            affine_arr[:, ln * 2 * NCH + NCH: (ln + 1) * 2 * NCH] = bb

    sh = _prep_shared(inputs)
    # core -> (program key, in_map)
    core_keys, core_maps = [], []
    for c in range(8):
        b, qh = c // 2, c % 2
        kts_cc = max(1, -(-int(inputs["source_code_len"][b]) // P))
        kts_ct = max(1, -(-int(inputs["template_len"][b]) // P))
        key = (qh, kts_cc, kts_ct, gate_b, aff)
        m = _prep_core(inputs, sh, b, qh, kts_cc, kts_ct)
        if aff:
            m["ln_affine"] = affine_arr
        core_keys.append(key)
        core_maps.append(m)

    # build distinct programs (parallel threads: walrus compile is subprocess)
    distinct = sorted(set(core_keys))
    threads = [threading.Thread(target=_get_program, args=(k,))
               for k in distinct]
    for t in threads:
        t.start()
    for t in threads:
        t.join()

    groups = []
    for key in distinct:
        cores = [c for c in range(8) if core_keys[c] == key]
        groups.append((_get_program(key), cores, [core_maps[c] for c in cores]))

    results = _run_groups(groups)

    out = np.empty((B, T, D), np.float32)
    for c in range(8):
        b, qh = c // 2, c % 2
        out[b, qh * 512:(qh + 1) * 512, :] = results[c]["outT"].T
    return out



# revision 42
# speedup vs baseline: 1.1358x; 1.0108x over previous
"""Trainium2 Bass kernel for nn_DecoderBlockWithKeywords.

Decoder block: causal self-attn + gated (source-code / keywords) cross-attn
+ template cross-attn + FFN, with 4 LayerNorms.  B=4, T=1024, D=512, H=8,
dh=64, DFF=2048.

Sharding: pure data-parallel over (batch, query-half) -> 8 NeuronCores, no
collectives.  Each core holds all weights (fp16) and computes 512 query
tokens of one batch element.

Layout strategy: every activation lives feature-major (X^T: [D on
partitions, tokens on free]).  Host pre-transposes/casts inputs.  Q/K
projections are weight-stationary (out feature-major); V is produced
token-major via activation-stationary matmuls so the attention AV matmul
needs no transposes at all.  Scores are computed as S^T = K_h Q_h^T
([kv, q]); softmax runs without max-subtraction (logits are O(1); masked
lanes get -1e6 bias fused into the ACT exp).  Softmax denominators come
from a ones-column appended to V inside the same AV matmul; per-column
scales (softmax 1/n, LN mean/rstd, gate g) are broadcast across partitions
with a PE ones-outer-product into a free PSUM bank and applied by one DVE
op reading that PSUM operand.  LayerNorm is done feature-major: column sums
via PE ones-matmuls, rstd = exp(-0.5*ln(v)) on ACT (single activation-table
set, zero table switches).  Residuals follow the reference post-LN chaining
(z = LN(y + y2), z_end = LN(z + z2), out = LN(z_end + ff)).

Programs are specialized at build time to the actual kv lengths (read from
the int32 length inputs), so masked kv tiles are skipped entirely; up to 8
distinct programs (4 batches x even/odd query half) are compiled and
launched concurrently on disjoint device subsets.
"""

import os
import sys
import threading

import numpy as np

for _p in ("/opt/trn_rl_repo", "/root/.axon_site"):
    if os.path.isdir(_p) and _p not in sys.path:
        sys.path.append(_p)

import ml_dtypes
from contextlib import ExitStack

import concourse.bass as bass
import concourse.mybir as mybir
from concourse import bacc
from concourse.tile import TileContext

BF16 = np.float16
F32 = np.float32
NEG = -1000000.0
B, T, S, TM, KW, D, H, DFF = 4, 1024, 1024, 512, 64, 512, 8, 2048
DH = D // H  # 64
P = 128
NCH = D // P  # 4 feature chunks
AF = mybir.ActivationFunctionType
OP = mybir.AluOpType


# ---------------------------------------------------------------------------
# program builder
# ---------------------------------------------------------------------------

def build_program(mode, kts_cc, kts_ct, gate_b=0.0, apply_affine=False,
                  debug=False):
    """Build one core's Bass program.

    mode 'a': q blocks [0:256]+[768:1024], kv 1024 (split causal spec);
    mode 'b': q block [256:768], kv 768 (standard causal).  Both cores
    see ~equal causal score/exp work.  kts_cc/kts_ct: number of 128-wide
    kv tiles for the source-code / template cross attentions.
    """
    f32, bf16 = mybir.dt.float32, mybir.dt.float16
    KV = 1024 if mode == "a" else 768   # self-attn kv range
    if mode == "a":
        # (matmul col start, stair block or None) per kv tile
        SA_SPEC = [(0, 0), (P, 1), (256, None), (256, None),
                   (256, None), (256, None), (256, 2), (384, 3)]
    else:
        SA_SPEC = [(0, None), (0, None), (0, 0), (P, 1), (256, 2), (384, 3)]

    nc = bacc.Bacc("TRN2", target_bir_lowering=False, debug=False)

    def din(name, shape, dt=bf16):
        return nc.dram_tensor(name, shape, dt, kind="ExternalInput").ap()

    fp8 = mybir.dt.float8e4
    xkv8T = din("xkv8T", [D, KV], fp8)
    xq8T = din("xq8T", [D, 512], fp8)
    src8T = din("src8T", [D, kts_cc * P], fp8)
    tmpl8T = din("tmpl8T", [D, kts_ct * P], fp8)
    kw8T = din("kw8T", [D, KW], fp8)
    xqT = din("xqT", [D, 512])
    W8NAMES = ("sa_wk", "sa_wv", "sa_wq", "cc_wk", "cc_wv",
               "ck_wk", "ck_wv", "ct_wk", "ct_wv")
    wnames = [f"{n}_{p}" for n in ("sa", "cc", "ct", "ck")
              for p in ("wq", "wk", "wv", "wo")]
    wd = {n: din(n, [D, D], fp8 if n in W8NAMES else bf16)
          for n in wnames}
    w1d = din("ffn_w1", [D, DFF])
    w2d = din("ffn_w2", [DFF, D])
    gwA = din("gwA", [D, 1])
    gwB = din("gwB", [D, 1])
    staird = din("stair", [P, P])
    ccbias_d = din("cc_bias", [P, 1], f32)
    ctbias_d = din("ct_bias", [P, 1], f32)
    kwbias_d = din("kw_bias", [KW, 1], f32)
    affine_d = din("ln_affine", [P, NCH * 8], f32) if apply_affine else None
    outT = nc.dram_tensor("outT", [D, 512], bf16, kind="ExternalOutput").ap()
    dbg_outs = {}

    def mkdbg(nm, shape):
        if nm not in dbg_outs:
            dbg_outs[nm] = nc.dram_tensor(f"dbg_{nm}", shape, f32,
                                          kind="ExternalOutput").ap()
        return dbg_outs[nm]

    with TileContext(nc, pool_alloc_mode="queue") as tc, ExitStack() as ctx:
        # Pin the activation table to natural_log_exp_and_others (set 6):
        # it contains exp/ln/square/relu/copy/identity, i.e. every ACT
        # function this program uses, so no further table loads are needed.
        nc.scalar.add_instruction(mybir.InstLoadActFuncSet(
            name=nc.get_next_instruction_name(), act_func_set_id=6,
            ins=[], outs=[]))
        pers = ctx.enter_context(tc.tile_pool(name="pers", bufs=1))
        # ---- persistent small constants -------------------------------
        stair = pers.tile([P, P], bf16, name="stair_t")
        nc.gpsimd.dma_start(out=stair, in_=staird)
        # selector for the head-pair 1/n broadcast: row 0 -> partitions
        # 0:64 (even head), row 32 -> partitions 64:128 (odd head)
        # head-pair 1/n machinery: denominators for pairs (0,1) land on
        # rows 0/32/64/96 of tile A, pairs (2,3) on tile B (32-aligned
        # partition writes only).  One DVE reciprocal+cast per tile.
        # selAB col block hp%2 maps rows (0,32) or (64,96) onto the
        # 64-partition halves of a pair's output.
        selAB = pers.tile([97, 2 * P], bf16, name="selAB_t")
        nc.vector.memset(selAB, 0.0)
        nc.gpsimd.memset(selAB[0:1, 0:DH], 1.0)
        nc.gpsimd.memset(selAB[32:33, DH:P], 1.0)
        nc.gpsimd.memset(selAB[64:65, P:P + DH], 1.0)
        nc.gpsimd.memset(selAB[96:97, P + DH:2 * P], 1.0)
        nden97 = [pers.tile([97, 512], f32, name=f"nden97_{i}")
                  for i in range(2)]
        for t in nden97:
            nc.vector.memset(t, 1.0)
        nrec97 = [pers.tile([97, 512], f32, name=f"nrec97_{i}")
                  for i in range(2)]
        ninv97 = [pers.tile([97, 512], bf16, name=f"ninv97_{i}")
                  for i in range(2)]
        ccbias = pers.tile([P, 1], f32, name="ccbias_t")
        nc.gpsimd.dma_start(out=ccbias, in_=ccbias_d)
        ctbias = pers.tile([P, 1], f32, name="ctbias_t")
        nc.gpsimd.dma_start(out=ctbias, in_=ctbias_d)
        kwbias = pers.tile([KW, 1], f32, name="kwbias_t")
        nc.gpsimd.dma_start(out=kwbias, in_=kwbias_d)
        gwa_t = pers.tile([P, NCH], bf16, name="gwa_t")
        nc.gpsimd.dma_start(out=gwa_t,
                            in_=gwA.rearrange("(i p) o -> p i o", p=P))
        gwb_t = pers.tile([P, NCH], bf16, name="gwb_t")
        nc.gpsimd.dma_start(out=gwb_t,
                            in_=gwB.rearrange("(i p) o -> p i o", p=P))
        ones_b = pers.tile([P, 1], bf16, name="ones_b")
        nc.vector.memset(ones_b, 1.0)
        ones_row = pers.tile([1, P], bf16, name="ones_row")
        nc.vector.memset(ones_row, 1.0)
        eps_t = pers.tile([1, 1], f32, name="eps_t")
        nc.vector.memset(eps_t, 1e-5)
        gb_t = pers.tile([1, 1], f32, name="gb_t")
        nc.vector.memset(gb_t, -float(gate_b))
        affine = None
        if apply_affine:
            affine = pers.tile([P, NCH * 8], f32, name="affine_t")
            nc.sync.dma_start(out=affine, in_=affine_d)

        def tap(nm, tiles):
            if not debug:
                return
            cols = tiles[0].shape[-1]
            d = mkdbg(nm, [len(tiles) * P, cols])
            for i, t in enumerate(tiles):
                rows = t.shape[0]
                nc.gpsimd.dma_start(out=d[i * P:i * P + rows, :], in_=t)

        # ---- global shared pools --------------------------------------
        # residual/LN-out tiles, reused across stages via shared tags
        rpool = ctx.enter_context(tc.tile_pool(name="rpool", bufs=1))

        def mktiles(nm, cols=512, dt=f32, n=NCH, tagp=None):
            tagp = tagp or nm
            return [rpool.tile([P, cols], dt, name=f"{nm}{i}", tag=f"{tagp}{i}",
                               bufs=1) for i in range(n)]

        # small 1/8-partition tiles + broadcast tiles, shared by all stages
        smallp = ctx.enter_context(tc.tile_pool(name="smallp", bufs=1))
        # transient [128, *] tiles (exp outputs, LN scratch, gate scratch)
        trp = ctx.enter_context(tc.tile_pool(name="trp", bufs=1))
        # PSUM: pps = projection/V accumulators; x_ps = paired scores
        # (2 banks each); x_po = AV out + LN stats + gate
        psA = ctx.enter_context(tc.tile_pool(name="psA", bufs=2, space="PSUM"))
        psB = ctx.enter_context(tc.tile_pool(name="psB", bufs=2, space="PSUM"))

        def load_w(pool, names):
            for n in names:
                if n in W8NAMES:
                    fp8 = mybir.dt.float8e4
                    wt[n] = pool.tile([P, 4 * D], fp8, name=f"{n}_t",
                                      tag=f"{n}_t", bufs=1)
                    nc.sync.dma_start(
                        out=wt[n].rearrange("p (c o n) -> p c o n",
                                            c=2, o=2),
                        in_=wd[n].rearrange("(c o p) n -> p c o n",
                                            c=2, o=2, p=P))
                else:
                    wt[n] = pool.tile([P, NCH * D], bf16, name=f"{n}_t",
                                      tag=f"{n}_t", bufs=1)
                    nc.sync.dma_start(
                        out=wt[n].rearrange("p (i n) -> p i n", n=D),
                        in_=wd[n].rearrange("(i p) n -> p i n", p=P))
        wt = {}

        def w8_lhsT(n, c, j):
            return wt[n].rearrange("p (c o n) -> p c o n",
                                   c=2, o=2)[:, c, :, j * P:(j + 1) * P]

        def w8_rhs(n, c):
            return wt[n].rearrange("p (c o n) -> p c o n", c=2, o=2)[:, c]

        def w8_flat(n, c, o, j):
            return wt[n].rearrange("p (c o n) -> p c o n",
                                   c=2, o=2)[:, c, o, j * P:(j + 1) * P]

        def load_act8(pool, nm, dram_ap, cols):
            fp8 = mybir.dt.float8e4
            tiles = []
            for c in range(2):
                t = pool.tile([P, 2 * cols], fp8, name=f"{nm}{c}",
                              tag=f"{nm}{c}", bufs=1)
                nc.sync.dma_start(
                    out=t.rearrange("p (o n) -> p o n", o=2),
                    in_=dram_ap[c * 2 * P:(c + 1) * 2 * P, :].rearrange(
                        "(o p) n -> p o n", o=2, p=P))
                tiles.append(t.rearrange("p (o n) -> p o n", o=2))
            return tiles

        def w_lhsT(n, i, j):
            return wt[n][:, i * D + j * P: i * D + (j + 1) * P]

        def w_rhs(n, i, cols=D):
            return wt[n][:, i * D: i * D + cols]

        def load_act(pool, nm, dram_ap, cols):
            tiles = []
            for i in range(NCH):
                t = pool.tile([P, cols], bf16, name=f"{nm}{i}",
                              tag=f"{nm}{i}", bufs=1)
                nc.sync.dma_start(out=t, in_=dram_ap[i * P:(i + 1) * P, :])
                tiles.append(t)
            return tiles

        # ----------------------------------------------------------------
        # helpers
        # ----------------------------------------------------------------
        def proj_fm_groups(wn, rhs_tiles, ncols, out_tiles, evict):
            """Per-psum-group closures for a feature-major projection; each
            emits 4 accumulating matmuls + one eviction."""
            ntt = (ncols + 511) // 512
            groups = []
            for j in range(NCH):
                for t in range(ntt):
                    def g(j=j, t=t):
                        cs = t * 512
                        ce = min(ncols, cs + 512)
                        ps = psA.tile([P, ce - cs], mybir.dt.float32,
                                      name="proj_ps", tag="pps")
                        for i in range(NCH):
                            nc.tensor.matmul(ps, w_lhsT(wn, i, j),
                                             rhs_tiles[i][:, cs:ce],
                                             start=(i == 0),
                                             stop=(i == NCH - 1))
                        evict(j, cs, ce, ps, out_tiles)
                    groups.append(g)
            return groups

        def proj_fm(wn, rhs_tiles, ncols, out_tiles, evict):
            for g in proj_fm_groups(wn, rhs_tiles, ncols, out_tiles, evict):
                g()

        DR = mybir.MatmulPerfMode.DoubleRow
        DESC = 1.0 / 1024.0  # descale: activation x16, weight x64

        def evict_ds(j, cs, ce, ps, out_tiles, balance=False):
            if balance and (j + cs // 512) % 2 == 1:
                nc.scalar.mul(out_tiles[j][:, cs:ce], ps, DESC)
            else:
                nc.vector.tensor_scalar_mul(out_tiles[j][:, cs:ce], ps, DESC)

        def proj_fm8_groups(wn, rhs8, ncols, out_tiles, balance=False):
            """fp8 DoubleRow feature-major projection (descaled evict).
            Falls back to normal-mode fp8 matmuls when ncols < 128."""
            ntt = (ncols + 511) // 512
            groups = []
            for j in range(NCH):
                for t in range(ntt):
                    def g(j=j, t=t):
                        cs = t * 512
                        ce = min(ncols, cs + 512)
                        ps = psA.tile([P, ce - cs], mybir.dt.float32,
                                      name="proj_ps", tag="pps")
                        if ncols >= P:
                            for c in range(2):
                                nc.tensor.matmul(
                                    ps, w8_lhsT(wn, c, j),
                                    rhs8[c][:, :, cs:ce],
                                    start=(c == 0), stop=(c == 1),
                                    perf_mode=DR)
                        else:
                            for ci in range(4):
                                c, o = ci // 2, ci % 2
                                nc.tensor.matmul(
                                    ps, w8_flat(wn, c, o, j),
                                    rhs8[c][:, o, cs:ce],
                                    start=(ci == 0), stop=(ci == 3))
                        evict_ds(j, cs, ce, ps, out_tiles, balance)
                    groups.append(g)
            return groups

        def proj_v8_groups(enc8, wn, nkv, vt_list, vpool, ktag):
            nch_tok = (nkv + P - 1) // P
            vt_list.extend(
                vpool.tile([min(P, nkv - m * P), H * (DH + 1)], bf16,
                           name=f"{ktag}_v{m}", tag=f"{ktag}_v{m}", bufs=1)
                for m in range(nch_tok))
            groups = []
            for m in range(nch_tok):
                def g(m=m):
                    rows = vt_list[m].shape[0]
                    ps = psA.tile([rows, D], mybir.dt.float32,
                                  name="v_ps", tag="pps")
                    for c in range(2):
                        nc.tensor.matmul(ps,
                                         enc8[c][:, :, m * P:m * P + rows],
                                         w8_rhs(wn, c),
                                         start=(c == 0), stop=(c == 1),
                                         perf_mode=DR)
                    vt = vt_list[m]
                    src3 = ps.rearrange("p (g c) -> p g c", c=DH)
                    dst3 = vt.rearrange("p (g c) -> p g c", c=DH + 1)
                    nc.vector.tensor_scalar_mul(dst3[:, :, 0:DH], src3, DESC)
                    nc.gpsimd.memset(dst3[:, :, DH:DH + 1], 1.0)
                groups.append(g)
            return groups

        def evict_copy(j, cs, ce, ps, out_tiles):
            nc.vector.tensor_copy(out_tiles[j][:, cs:ce], ps)

        def evict_copy_bal(j, cs, ce, ps, out_tiles):
            if (j + cs // 512) % 2 == 0:
                nc.vector.tensor_copy(out_tiles[j][:, cs:ce], ps)
            else:
                nc.scalar.copy(out_tiles[j][:, cs:ce], ps)

        def proj_v_groups(enc_tiles, wn, nkv, vt_list, vpool, ktag):
            nch_tok = (nkv + P - 1) // P
            vt_list.extend(
                vpool.tile([min(P, nkv - m * P), H * (DH + 1)], bf16,
                           name=f"{ktag}_v{m}", tag=f"{ktag}_v{m}", bufs=1)
                for m in range(nch_tok))
            groups = []
            for m in range(nch_tok):
                def g(m=m):
                    rows = vt_list[m].shape[0]
                    ps = psA.tile([rows, D], mybir.dt.float32,
                                  name="v_ps", tag="pps")
                    for i in range(NCH):
                        nc.tensor.matmul(ps,
                                         enc_tiles[i][:, m * P:m * P + rows],
                                         w_rhs(wn, i),
                                         start=(i == 0), stop=(i == NCH - 1))
                    vt = vt_list[m]
                    src3 = ps.rearrange("p (g c) -> p g c", c=DH)
                    dst3 = vt.rearrange("p (g c) -> p g c", c=DH + 1)
                    nc.vector.tensor_copy(dst3[:, :, 0:DH], src3)
                    nc.gpsimd.memset(dst3[:, :, DH:DH + 1], 1.0)
                groups.append(g)
            return groups

        def proj_v(enc_tiles, wn, nkv, vt_list, vpool, ktag):
            for g in proj_v_groups(enc_tiles, wn, nkv, vt_list, vpool, ktag):
                g()

        def attention(qt, kt, vt_list, out_tiles, bias_tile, cspec, ktag,
                      fillers=None):
            """Multi-head attention.  Head pairs share one [rows,1024]
            scores psum + one merged exp; the AV matmul for tile kt is
            emitted after the scores matmul for tile kt+1 so the ACT exp
            overlaps PE work.  Causal scores/AV are restricted to the
            unmasked column range.  The softmax normalization runs per
            head-pair (collect denominators on partitions 0/32, one DVE
            reciprocal, one K=33 selector broadcast, one fused multiply)
            so it hides under the next pair's kt loop.  `fillers` is a
            list of closures emitting independent PE work; one is popped
            after each kt iteration to fill the exp-wait bubbles."""
            nkt = len(vt_list)
            fillers = fillers if fillers is not None else []

            def pop_filler():
                if fillers:
                    fillers.pop(0)()

            for hp in range(H // 2):
                po = []
                for s in range(2):
                    po.append(psB.tile([DH + 1, 512], mybir.dt.float32,
                                       name=f"{ktag}_po{s}", tag="x_po"))
                pend = None  # deferred AV: (kt_i, pt2, c0)

                def flush_av(last):
                    kt_i, pt2, c0 = pend
                    for s in range(2):
                        h = 2 * hp + s
                        nc.tensor.matmul(
                            po[s][:, c0:512],
                            vt_list[kt_i][:, h * (DH + 1):
                                          (h + 1) * (DH + 1)],
                            pt2[:, s * 512 + c0:(s + 1) * 512],
                            start=(kt_i == 0), stop=last)

                for kt_i in range(nkt):
                    rows = vt_list[kt_i].shape[0]
                    c0, spos = cspec[kt_i] if cspec else (0, None)
                    ps2 = psB.tile([rows, 1024], mybir.dt.float32,
                                   name=f"{ktag}_ps", tag="x_ps")
                    pt2 = trp.tile([rows, 1024], bf16,
                                   name=f"{ktag}_pt", tag="pt", bufs=3)
                    for s in range(2):
                        ro = s * DH
                        o = s * 512
                        nc.tensor.matmul(
                            ps2[:, o + c0:o + 512],
                            kt[hp][ro:ro + DH, kt_i * P:kt_i * P + rows],
                            qt[hp][ro:ro + DH, c0:512], start=True, stop=True)
                    if cspec and (c0 > 0 or spos is not None):
                        for s in range(2):
                            o = s * 512
                            nc.scalar.activation(pt2[:, o + c0:o + 512],
                                                 ps2[:, o + c0:o + 512],
                                                 AF.Exp, scale=0.125)
                            if spos is not None:
                                nc.vector.tensor_mul(
                                    pt2[:, o + spos * P:o + (spos + 1) * P],
                                    pt2[:, o + spos * P:o + (spos + 1) * P],
                                    stair)
                    elif cspec:
                        nc.scalar.activation(pt2, ps2, AF.Exp, scale=0.125)
                    else:
                        bias = 0.0
                        if bias_tile is not None and kt_i == nkt - 1:
                            bias = bias_tile[:rows, :]
                        nc.scalar.activation(pt2, ps2, AF.Exp,
                                             bias=bias, scale=0.125)
                    if pend is not None:
                        flush_av(False)
                        if kt_i % 2 == 1:
                            pop_filler()
                    pend = (kt_i, pt2, c0)
                flush_av(True)
                # stage this pair's denominators into tile A (pairs 0,1)
                # or B (pairs 2,3) on 32-aligned partitions
                ab, r0 = hp // 2, (hp % 2) * DH
                nc.vector.tensor_copy(nden97[ab][r0:r0 + 1, :],
                                      po[0][DH:DH + 1, :])
                nc.vector.tensor_copy(nden97[ab][r0 + 32:r0 + 33, :],
                                      po[1][DH:DH + 1, :])
                nc.scalar.copy(out_tiles[hp][0:DH, :], po[0][0:DH, :])
                nc.vector.tensor_copy(out_tiles[hp][DH:P, :], po[1][0:DH, :])
                if hp % 2 == 1:
                    # both pairs of this tile staged: one reciprocal+cast,
                    # then normalize both pairs (overlaps the next loop)
                    nc.vector.reciprocal_approx_fast(out=nrec97[ab],
                                                     in_=nden97[ab])
                    nc.vector.tensor_copy(ninv97[ab], nrec97[ab])
                    for hq in (hp - 1, hp):
                        pop_filler()
                        nb = psA.tile([P, 512], mybir.dt.float32,
                                      name=f"{ktag}_nb{hq}", tag="pps")
                        nc.tensor.matmul(nb,
                                         selAB[:, (hq % 2) * P:
                                               (hq % 2 + 1) * P],
                                         ninv97[ab], start=True, stop=True)
                        nc.vector.tensor_mul(out_tiles[hq], out_tiles[hq],
                                             nb)
            for g in fillers:
                g()

        def layernorm(r_tiles, out_tiles, ln_idx, mid=None):
            sq = [trp.tile([P, 512], bf16, name=f"ln{ln_idx}_sq", tag="ln_sq",
                           bufs=2) for _ in range(NCH)]
            for j in range(NCH):
                nc.scalar.activation(sq[j], r_tiles[j], AF.Square)
            ps_s = psB.tile([1, 512], mybir.dt.float32,
                            name="ln_ps_s", tag="x_po")
            ps_q = psB.tile([1, 512], mybir.dt.float32,
                            name="ln_ps_q", tag="x_po")
            for j in range(NCH):
                nc.tensor.matmul(ps_s, ones_b, r_tiles[j],
                                 start=(j == 0), stop=(j == NCH - 1))
            for j in range(NCH):
                nc.tensor.matmul(ps_q, ones_b, sq[j],
                                 start=(j == 0), stop=(j == NCH - 1))
            if mid is not None:
                mid()
            mean16 = smallp.tile([1, 512], bf16,
                                 name="ln_mean16", tag="ln_stat", bufs=3)
            nc.vector.tensor_scalar_mul(mean16, ps_s, 1.0 / D)
            meanb = psB.tile([P, 512], mybir.dt.float32,
                             name="ln_meanb", tag="x_po")
            nc.tensor.matmul(meanb, ones_row, mean16, start=True, stop=True)
            msq = smallp.tile([1, 512], mybir.dt.float32,
                              name="ln_msq", tag="ln_stat", bufs=3)
            nc.scalar.activation(msq, ps_s, AF.Square, scale=1.0 / D)
            var = smallp.tile([1, 512], mybir.dt.float32,
                              name="ln_var", tag="ln_stat", bufs=3)
            nc.vector.scalar_tensor_tensor(var, ps_q, 1.0 / D, msq,
                                           op0=OP.mult, op1=OP.subtract)
            lnv = smallp.tile([1, 512], mybir.dt.float32,
                              name="ln_lnv", tag="ln_stat", bufs=3)
            nc.scalar.activation(lnv, var, AF.Ln, bias=eps_t[:, :])
            rstd = smallp.tile([1, 512], bf16,
                               name="ln_rstd", tag="ln_stat", bufs=3)
            nc.scalar.activation(rstd, lnv, AF.Exp, scale=-0.5)
            rstdb = psB.tile([P, 512], mybir.dt.float32,
                             name="ln_rstdb", tag="x_po")
            nc.tensor.matmul(rstdb, ones_row, rstd, start=True, stop=True)
            for j in range(NCH):
                tmp = trp.tile([P, 512], bf16,
                               name="ln_tmp", tag="ln_tmp", bufs=2)
                nc.vector.tensor_sub(tmp, r_tiles[j], meanb)
                nc.vector.tensor_mul(out_tiles[j], tmp, rstdb)
                if apply_affine:
                    g = affine[:, ln_idx * 2 * NCH + j: ln_idx * 2 * NCH + j + 1]
                    b = affine[:, ln_idx * 2 * NCH + NCH + j:
                               ln_idx * 2 * NCH + NCH + j + 1]
                    nc.vector.tensor_scalar(out_tiles[j], out_tiles[j],
                                            g, b, op0=OP.mult, op1=OP.add)

        # ================================================================
        # emission (ordered for cross-stage overlap)
        # ================================================================
        r1 = mktiles("r1", dt=bf16, tagp="rA")
        y = mktiles("y", dt=bf16, tagp="lnA")
        r2 = mktiles("r2", dt=bf16, tagp="rB")
        z = mktiles("z", dt=bf16, tagp="lnB")
        r3 = None  # allocated after r1 dies
        ze = None

        # ct pool created first so it outlives ccsb (LIFO pool stack);
        # its DMA loads are issued after LN1 and overlap the cc/ck stage
        ctsb = ctx.enter_context(tc.tile_pool(name="tail_sb", bufs=1))
        ccsb_cm = tc.tile_pool(name="cc_sb", bufs=1)
        ccsb = ccsb_cm.__enter__()
        sasb_cm = tc.tile_pool(name="sa_sb", bufs=1)
        sasb = sasb_cm.__enter__()

        # --- stage 1: self attention (fp8 DoubleRow K/V/Q) ---
        load_w(sasb, ["sa_wk"])
        xkv8 = load_act8(sasb, "xkv8", xkv8T, KV)
        load_w(sasb, ["sa_wv", "sa_wq"])
        xq = load_act(sasb, "xq", xqT, 512)
        load_w(sasb, ["sa_wo"])
        qt = [sasb.tile([P, 512], bf16, name=f"sa_q{i}", tag=f"sa_q{i}",
                        bufs=1) for i in range(NCH)]
        ktl = [sasb.tile([P, KV], bf16, name=f"sa_k{i}", tag=f"sa_k{i}",
                         bufs=1) for i in range(NCH)]
        xq8 = load_act8(sasb, "xq8", xq8T, 512)
        for g in proj_fm8_groups("sa_wk", xkv8, KV, ktl, balance=True):
            g()
        vts = []
        for g in proj_v8_groups(xkv8, "sa_wv", KV, vts, sasb, "sa"):
            g()
        for g in proj_fm8_groups("sa_wq", xq8, 512, qt, balance=True):
            g()
        at = [trp.tile([P, 512], bf16, name=f"sa_at{i}", tag=f"at{i}",
                       bufs=1) for i in range(NCH)]
        # cc K/V projections are independent of sa: interleave them into
        # sa's kt loops as PE fillers (their DMA loads were issued above)
        load_w(ccsb, ["cc_wk", "cc_wv", "ck_wk", "ck_wv",
                      "cc_wq", "ck_wq", "cc_wo", "ck_wo"])
        srcl8 = load_act8(ccsb, "src8", src8T, kts_cc * P)
        kwe8 = load_act8(ccsb, "kw8", kw8T, KW)
        cc_kt = [ccsb.tile([P, kts_cc * P], bf16, name=f"cc_k{i}",
                           tag=f"cc_k{i}", bufs=1) for i in range(NCH)]
        cc_vts = []
        sa_fill = (proj_fm8_groups("cc_wk", srcl8, kts_cc * P, cc_kt)
                   + proj_v8_groups(srcl8, "cc_wv", kts_cc * P, cc_vts,
                                    ccsb, "cc"))
        attention(qt, ktl, vts, at, None, SA_SPEC, "sa",
                  fillers=sa_fill)

        def evict_resid_x(j, cs, ce, ps, out_tiles):
            nc.vector.tensor_add(out_tiles[j][:, cs:ce], ps, xq[j])
        tap("sa_at", at)
        proj_fm("sa_wo", at, 512, r1, evict_resid_x)
        tap("r1", r1)
        ck_kt = [ccsb.tile([P, KW], bf16, name=f"ck_k{i}", tag=f"ck_k{i}",
                           bufs=1) for i in range(NCH)]
        ck_vts = []

        def ln1_mid():
            for g in proj_fm8_groups("ck_wk", kwe8, KW, ck_kt):
                g()
            for g in proj_v8_groups(kwe8, "ck_wv", KW, ck_vts, ccsb, "ck"):
                g()
        layernorm(r1, y, 0, mid=ln1_mid)
        tap("y", y)
        sasb_cm.__exit__(None, None, None)

        # ct weight/activation DMA loads overlap the whole cc/ck stage
        load_w(ctsb, ["ct_wk", "ct_wv", "ct_wq", "ct_wo"])
        tmpl8 = load_act8(ctsb, "tmpl8", tmpl8T, kts_ct * P)

        # --- stage 2: cc + ck cross attention + gate ---
        cc_qt = [ccsb.tile([P, 512], bf16, name=f"cc_q{i}", tag=f"cc_q{i}",
                           bufs=1) for i in range(NCH)]
        proj_fm("cc_wq", y, 512, cc_qt, evict_copy_bal)
        cc_at = [trp.tile([P, 512], bf16, name=f"cc_at{i}", tag=f"at{i}",
                          bufs=1) for i in range(NCH)]
        ck_qt = [ccsb.tile([P, 512], bf16, name=f"ck_q{i}", tag=f"ck_q{i}",
                           bufs=1) for i in range(NCH)]
        cc_fill = proj_fm_groups("ck_wq", y, 512, ck_qt, evict_copy)
        attention(cc_qt, cc_kt, cc_vts, cc_at, ccbias, None, "cc",
                  fillers=cc_fill)
        ck_at = [trp.tile([P, 512], bf16, name=f"ck_at{i}", tag=f"ckat{i}",
                          bufs=1) for i in range(NCH)]
        y2c = [ccsb.tile([P, 512], bf16, name=f"y2c{i}", tag=f"y2c{i}",
                         bufs=1) for i in range(NCH)]
        ck_fill = proj_fm_groups("cc_wo", cc_at, 512, y2c, evict_copy)
        attention(ck_qt, ck_kt, ck_vts, ck_at, kwbias, None, "ck",
                  fillers=ck_fill)
        y2k = [ccsb.tile([P, 512], bf16, name=f"y2k{i}", tag=f"y2k{i}",
                         bufs=1) for i in range(NCH)]
        proj_fm("ck_wo", ck_at, 512, y2k, evict_copy_bal)

        # --- gate ---
        ps_g = psB.tile([1, 512], mybir.dt.float32, name="gate_ps",
                        tag="x_po")
        for i in range(NCH):
            nc.tensor.matmul(ps_g, gwa_t[:, i:i + 1], y2c[i],
                             start=(i == 0), stop=False)
        for i in range(NCH):
            nc.tensor.matmul(ps_g, gwb_t[:, i:i + 1], y2k[i],
                             start=False, stop=(i == NCH - 1))
        # g-independent combine pieces, overlap the gate ACT/DVE chain
        gdt = [trp.tile([P, 512], bf16, name=f"gate_dt{j}", tag=f"gate_dt{j}",
                        bufs=1) for j in range(NCH)]
        for j in range(NCH):
            nc.vector.tensor_sub(gdt[j], y2c[j], y2k[j])
            nc.vector.tensor_add(r2[j], y[j], y2k[j])
        ct_kt = [ctsb.tile([P, kts_ct * P], bf16, name=f"ct_k{i}",
                           tag=f"ct_k{i}", bufs=1) for i in range(NCH)]

        def ct_mid():
            for g in proj_fm8_groups("ct_wk", tmpl8, kts_ct * P, ct_kt):
                g()
        ge = smallp.tile([1, 512], mybir.dt.float32, name="gate_e",
                         tag="gate_edg", bufs=2)
        nc.scalar.activation(ge, ps_g, AF.Exp, scale=-1.0, bias=gb_t[:, :])
        gp1 = smallp.tile([1, 512], mybir.dt.float32, name="gate_p1",
                          tag="gate_edg", bufs=2)
        nc.vector.tensor_scalar_add(gp1, ge, 1.0)
        grc = smallp.tile([1, 512], mybir.dt.float32, name="gate_rc",
                          tag="gate_edg", bufs=2)
        nc.vector.reciprocal_approx_fast(out=grc, in_=gp1)
        gg = smallp.tile([1, 512], bf16, name="gate_g",
                         tag="gate_edg", bufs=2)
        nc.vector.tensor_copy(gg, grc)
        ct_mid()
        ggb = psB.tile([P, 512], mybir.dt.float32, name="gate_gb",
                       tag="x_po")
        nc.tensor.matmul(ggb, ones_row, gg, start=True, stop=True)
        # r2 = (y + y2k) + g*(y2c - y2k)
        for j in range(NCH):
            nc.vector.tensor_mul(gdt[j], gdt[j], ggb)
            nc.vector.tensor_add(r2[j], r2[j], gdt[j])
        tap("y2c", y2c)
        tap("y2k", y2k)
        tap("r2", r2)
        ccsb_cm.__exit__(None, None, None)
        # FFN weights: DMA overlaps the ct attention stage
        ffsb = ctx.enter_context(tc.tile_pool(name="ff_sb", bufs=1))
        w1t = ffsb.tile([P, NCH * DFF], bf16, name="w1_t", tag="w1_t")
        nc.sync.dma_start(out=w1t.rearrange("p (i n) -> p i n", n=DFF),
                          in_=w1d.rearrange("(i p) n -> p i n", p=P))
        w2t = ffsb.tile([P, (DFF // P) * D], bf16, name="w2_t", tag="w2_t")
        nc.sync.dma_start(out=w2t.rearrange("p (i n) -> p i n", n=D),
                          in_=w2d.rearrange("(i p) n -> p i n", p=P))
        ct_vts = []

        def ln2_mid():
            for g in proj_v8_groups(tmpl8, "ct_wv", kts_ct * P, ct_vts,
                                    ctsb, "ct"):
                g()
        layernorm(r2, z, 1, mid=ln2_mid)
        tap("z", z)

        # --- stage 3: ct cross attention ---
        r3 = mktiles("r3", dt=bf16, tagp="rA")
        ze = mktiles("ze", dt=bf16, tagp="lnA")
        ct_qt = [ffsb.tile([P, 512], bf16, name=f"ct_q{i}", tag=f"ct_q{i}",
                           bufs=1) for i in range(NCH)]
        proj_fm("ct_wq", z, 512, ct_qt, evict_copy_bal)
        ct_at = [trp.tile([P, 512], bf16, name=f"ct_at{i}", tag=f"at{i}",
                          bufs=1) for i in range(NCH)]
        attention(ct_qt, ct_kt, ct_vts, ct_at, ctbias, None, "ct")

        def evict_resid_r2(j, cs, ce, ps, out_tiles):
            nc.vector.tensor_add(out_tiles[j][:, cs:ce], ps, z[j])
        tap("ct_at", ct_at)
        proj_fm("ct_wo", ct_at, 512, r3, evict_resid_r2)
        tap("r3", r3)
        layernorm(r3, ze, 2)
        tap("ze", ze)

        # --- stage 4: FFN ---
        ht = [ffsb.tile([P, 512], bf16, name=f"ff_h{i}", tag=f"ff_h{i}",
                        bufs=1) for i in range(DFF // P)]
        for jf in range(DFF // P):
            ps = psA.tile([P, 512], mybir.dt.float32, name="ff_ps",
                          tag="pps")
            for i in range(NCH):
                nc.tensor.matmul(ps, w1t[:, i * DFF + jf * P:
                                         i * DFF + (jf + 1) * P],
                                 ze[i], start=(i == 0), stop=(i == NCH - 1))
            if jf % 2 == 0:
                nc.scalar.activation(ht[jf], ps, AF.Relu)
            else:
                nc.vector.tensor_scalar_max(ht[jf], ps, 0.0)
        r4 = mktiles("r4", dt=bf16, tagp="rB")
        for j in range(NCH):
            ps = psA.tile([P, 512], mybir.dt.float32, name="ff_ps2",
                          tag="pps")
            for i in range(DFF // P):
                nc.tensor.matmul(ps, w2t[:, i * D + j * P: i * D + (j + 1) * P],
                                 ht[i], start=(i == 0),
                                 stop=(i == DFF // P - 1))
            nc.vector.tensor_add(r4[j], ps, ze[j])
        fin = [trp.tile([P, 512], bf16, name=f"fin{i}",
                        tag=f"at{i}", bufs=1) for i in range(NCH)]
        layernorm(r4, fin, 3)
        for j in range(NCH):
            nc.sync.dma_start(out=outT[j * P:(j + 1) * P, :], in_=fin[j])

    nc.compile()
    return nc


# ---------------------------------------------------------------------------
# host-side input preparation
# ---------------------------------------------------------------------------

W8NAMES_H = ("sa_wk", "sa_wv", "sa_wq", "cc_wk", "cc_wv",
             "ck_wk", "ck_wv", "ct_wk", "ct_wv")
FP8 = ml_dtypes.float8_e4m3
SW8, SX8 = 64.0, 16.0


def _prep_shared(inputs):
    """Cast/transform weights shared by every core."""
    sh = {}
    for n in ("sa", "cc", "ct", "ck"):
        for p in ("wq", "wk", "wv", "wo"):
            nm = f"{n}_{p}"
            if nm in W8NAMES_H:
                sh[nm] = np.ascontiguousarray(
                    np.clip(inputs[nm].astype(F32) * SW8,
                            -240, 240).astype(FP8))
            else:
                sh[nm] = np.ascontiguousarray(inputs[nm].astype(BF16))
    sh["ffn_w1"] = np.ascontiguousarray(inputs["ffn_w1"].astype(BF16))
    sh["ffn_w2"] = np.ascontiguousarray(inputs["ffn_w2"].astype(BF16))
    gw = inputs["gate_w"].astype(F32)
    sh["gwA"] = np.ascontiguousarray(gw[:D].astype(BF16))
    sh["gwB"] = np.ascontiguousarray(gw[D:].astype(BF16))
    kl, ql = np.arange(P)[:, None], np.arange(P)[None, :]
    sh["stair"] = np.where(kl <= ql, 1.0, 0.0).astype(BF16)
    return sh


def _len_bias(L, kts, width=P):
    """[width,1] f32 additive bias for the LAST kv tile."""
    base = (kts - 1) * P
    idx = base + np.arange(width)
    return np.where(idx < L, 0.0, NEG).astype(F32)[:, None]


def _q8(a):
    return np.clip(a.astype(F32) * SX8, -240, 240).astype(FP8)


def _q_idx(mode):
    return (np.r_[0:256, 768:1024] if mode == "a" else np.r_[256:768])


def _prep_core(inputs, sh, b, mode, kts_cc, kts_ct):
    KVn = 1024 if mode == "a" else 768
    m = dict(sh)
    xT = inputs["x"][b].T.astype(F32)  # [D, T]
    qi = _q_idx(mode)
    m["xkv8T"] = np.ascontiguousarray(_q8(xT[:, :KVn]))
    m["xqT"] = np.ascontiguousarray(xT[:, qi].astype(BF16))
    m["xq8T"] = np.ascontiguousarray(_q8(xT[:, qi]))
    Ls = int(inputs["source_code_len"][b])
    st = np.zeros((D, kts_cc * P), FP8)
    st[:, :Ls] = _q8(inputs["source_code_enc"][b, :Ls].T)
    m["src8T"] = st
    Lt = int(inputs["template_len"][b])
    tt = np.zeros((D, kts_ct * P), FP8)
    tt[:, :Lt] = _q8(inputs["template_enc"][b, :Lt].T)
    m["tmpl8T"] = tt
    m["kw8T"] = np.ascontiguousarray(_q8(inputs["keywords_enc"][b].T))
    m["cc_bias"] = _len_bias(Ls, kts_cc)
    m["ct_bias"] = _len_bias(Lt, kts_ct)
    m["kw_bias"] = _len_bias(int(inputs["keywords_len"][b]), 1, KW)
    return m


# ---------------------------------------------------------------------------
# concurrent multi-program PJRT runner (adapted from bass2jax.run_bass_via_pjrt)
# ---------------------------------------------------------------------------

def _run_groups(groups):
    """groups: list of (nc, core_ids, in_maps).  Dispatch all groups onto
    their own device subsets, then gather.  Returns {core_id: {name: arr}}."""
    import jax
    import numpy as _np
    from jax.sharding import Mesh, PartitionSpec
    from jax.experimental.shard_map import shard_map
    from concourse import bass2jax
    from concourse.bass2jax import (_bass_exec_p, install_neuronx_cc_hook,
                                    partition_id_tensor)

    install_neuronx_cc_hook()
    devices = jax.devices()

    def make_launch(nc, core_ids, in_maps):
        pname = (nc.partition_id_tensor.name
                 if nc.partition_id_tensor else None)
        in_names, out_names, out_avals, zero_outs = [], [], [], []
        for alloc in nc.m.functions[0].allocations:
            if not isinstance(alloc, mybir.MemoryLocationSet):
                continue
            name = alloc.memorylocations[0].name
            if alloc.kind == "ExternalInput":
                if name == pname:
                    continue
                in_names.append(name)
            elif alloc.kind == "ExternalOutput":
                shape = tuple(alloc.tensor_shape)
                dtype = mybir.dt.np(alloc.dtype)
                out_names.append(name)
                out_avals.append(jax.core.ShapedArray(shape, dtype))
                zero_outs.append(_np.zeros(shape, dtype))
        n_params, n_outs = len(in_names), len(out_avals)
        all_in_names = in_names + out_names
        if pname is not None:
            all_in_names = all_in_names + [pname]

        def _body(*args):
            operands = list(args)
            if pname is not None:
                operands.append(partition_id_tensor())
            outs = _bass_exec_p.bind(
                *operands, out_avals=tuple(out_avals),
                in_names=tuple(all_in_names), out_names=tuple(out_names),
                lowering_input_output_aliases=(),
                sim_require_finite=False, sim_require_nnan=False, nc=nc)
            return tuple(outs)

        donate = tuple(range(n_params, n_params + n_outs))
        devs = [devices[c] for c in core_ids]
        if len(core_ids) == 1:
            fn = jax.jit(_body, donate_argnums=donate, keep_unused=True,
                         device=devs[0])
            args = [in_maps[0][nm] for nm in in_names] + list(zero_outs)
            out_arrs = fn(*args)
            return out_names, out_avals, out_arrs, None
        mesh = Mesh(_np.asarray(devs), ("core",))
        in_specs = (PartitionSpec("core"),) * (n_params + n_outs)
        out_specs = (PartitionSpec("core"),) * n_outs
        fn = jax.jit(shard_map(_body, mesh=mesh, in_specs=in_specs,
                               out_specs=out_specs, check_rep=False),
                     donate_argnums=donate, keep_unused=True)
        cat = [_np.concatenate([_np.asarray(m[nm]) for m in in_maps], axis=0)
               for nm in in_names]
        catz = [_np.zeros((len(core_ids) * z.shape[0], *z.shape[1:]), z.dtype)
                for z in zero_outs]
        out_arrs = fn(*cat, *catz)
        return out_names, out_avals, out_arrs, len(core_ids)

    last_err = None
    for _attempt in range(3):
        try:
            launched = []
            for nc, core_ids, in_maps in groups:
                launched.append((core_ids, make_launch(nc, core_ids, in_maps)))
            results = {}
            for core_ids, (out_names, out_avals, out_arrs, ncores) in launched:
                if ncores is None:
                    results[core_ids[0]] = {nm: _np.asarray(out_arrs[i])
                                            for i, nm in enumerate(out_names)}
                else:
                    for ci, c in enumerate(core_ids):
                        results[c] = {
                            nm: _np.asarray(out_arrs[i]).reshape(
                                ncores, *out_avals[i].shape)[ci]
                            for i, nm in enumerate(out_names)}
            return results
        except Exception as e:  # transient NRT device errors: retry
            last_err = e
            import time as _time
            _time.sleep(2.0)
    raise last_err


_PROGRAM_CACHE = {}
_CACHE_LOCK = threading.Lock()


def _get_program(key):
    with _CACHE_LOCK:
        if key in _PROGRAM_CACHE:
            return _PROGRAM_CACHE[key]
    mode, kts_cc, kts_ct, gate_b, aff = key
    nc = build_program(mode, kts_cc, kts_ct, gate_b=gate_b, apply_affine=aff)
    with _CACHE_LOCK:
        _PROGRAM_CACHE[key] = nc
    return nc


# ---------------------------------------------------------------------------
# entry point
# ---------------------------------------------------------------------------

def kernel(**inputs):
    inputs = {k: np.asarray(v) for k, v in inputs.items()}
    gate_b = float(inputs["gate_b"].reshape(-1)[0])
    aff = not all(
        np.all(inputs[f"ln{j}_g"] == 1.0) and np.all(inputs[f"ln{j}_b"] == 0.0)
        for j in range(1, 5))
    affine_arr = None
    if aff:
        affine_arr = np.zeros((P, NCH * 8), F32)
        for ln in range(4):
            g = inputs[f"ln{ln + 1}_g"].astype(F32).reshape(NCH, P).T
            bb = inputs[f"ln{ln + 1}_b"].astype(F32).reshape(NCH, P).T
            affine_arr[:, ln * 2 * NCH: ln * 2 * NCH + NCH] = g
            affine_arr[:, ln * 2 * NCH + NCH: (ln + 1) * 2 * NCH] = bb

    sh = _prep_shared(inputs)
    # core -> (program key, in_map)
    core_keys, core_maps = [], []
    for c in range(8):
        b, mode = c // 2, "ab"[c % 2]
        kts_cc = max(1, -(-int(inputs["source_code_len"][b]) // P))
        kts_ct = max(1, -(-int(inputs["template_len"][b]) // P))
        key = (mode, kts_cc, kts_ct, gate_b, aff)
        m = _prep_core(inputs, sh, b, mode, kts_cc, kts_ct)
        if aff:
            m["ln_affine"] = affine_arr
        core_keys.append(key)
        core_maps.append(m)

    # build distinct programs (parallel threads: walrus compile is subprocess)
    distinct = sorted(set(core_keys))
    threads = [threading.Thread(target=_get_program, args=(k,))
               for k in distinct]
    for t in threads:
        t.start()
    for t in threads:
        t.join()

    groups = []
    for key in distinct:
        cores = [c for c in range(8) if core_keys[c] == key]
        groups.append((_get_program(key), cores, [core_maps[c] for c in cores]))

    results = _run_groups(groups)

    out = np.empty((B, T, D), np.float32)
    for c in range(8):
        b, mode = c // 2, "ab"[c % 2]
        out[b, _q_idx(mode), :] = results[c]["outT"].T
    return out

